# revision 14
# baseline (speedup 1.0000x reference)
"""Trainium2 Bass kernel for nn_Decoder_57432302682540.

Strategy (pure data-parallel over batch, 8 NeuronCores):
  - Host: shard B=4096 into 8x512, pre-gather the recurrence rows
    (h_t, v_t, enc[idx]) and pre-transpose everything into [d, b]
    layout so the PE contracts over partitions.
  - Device (per core, all matmuls in fp32r = full-rate reduced-precision
    fp32): 5-step edge computation (both branches + predicated select),
    subtree max-chain via We, query construction (Wi, Wq), then the
    4-way tanh attention (Wc projection, +inp, tanh, V4-weighted
    reduction over h done as a [128,4] matmul) producing raw
    att[k, l, b] per core.
  - Host: mask, 10*tanh, softmax over the batch axis (the cross-shard
    coupling), categorical sampling with jax key 42 (gumbel-argmax,
    identical to jax.random.categorical), p gather and mask update.
    Rows whose sampling margin is within DELTA of a tie are recomputed
    exactly on the host (float64) so reduced-precision matmuls cannot
    flip an argmax.
"""

import numpy as np

B, L, D, H, T = 4096, 7, 1024, 1024, 6
NCORES = 8
BS = B // NCORES  # 512 rows per core
NSTEP = 5  # last scan step's edge/subtree never reaches the output
KD = D // 128  # 8 contraction chunks
KH = H // 128  # 8 output chunks
NC4 = 4 * H // 128  # 32 attention output chunks
DELTA = 0.30  # sampling-margin below which rows are recomputed on host
ASAT = 3.0  # |att_raw| at the chosen column below which p is recomputed

_PROG = None  # cached compiled Bass program


def _build_program():
    import concourse.bacc as bacc
    import concourse.mybir as mybir
    from concourse import tile

    F32 = mybir.dt.float32
    F32R = mybir.dt.float32r
    U8 = mybir.dt.uint8
    AF = mybir.ActivationFunctionType

    nc = bacc.Bacc()

    def inp(name, shape, dt=F32):
        return nc.declare_dram_parameter(name, shape, dt, isOutput=False)

    # weights are host-prepacked in consumption order: leading dim is the
    # output chunk the matmul loop consumes, so the first matmul group only
    # waits for its own chunk's DMA.
    encT = inp("encT", [L, D, BS], F32R)
    ghvT = inp("ghvT", [NSTEP, 2 * D, BS], F32R)  # per step: h then v chunks
    eidxT = inp("eidxT", [D, BS], F32R)
    cmask = inp("cmask", [NSTEP, 128, BS], U8)  # (t==0) as 0/1
    w4 = inp("w4", [KH, 128, 4, KD, 128], F32R)  # [m,p,(Wh,Wv,Wsh,Wsv),k,q]
    we4 = inp("we4", [KH, 128, KD, 128], F32R)  # [m,p,k,q] of We.T
    wi4 = inp("wi4", [KH, 128, KD, 128], F32R)
    wq4 = inp("wq4", [KH, 128, KD, 128], F32R)
    wc4 = inp("wc4", [NC4, 128, KD, 128], F32R)  # [c4,p,k,q] of Wc flat
    bi = inp("bi", [128, KH])
    btot = inp("btot", [128, NC4])  # bc (k-major) + bq, per chunk column
    v4s = inp("v4s", [128, NC4, 4], F32R)  # V4 chunk in column k of its group
    att = nc.declare_dram_parameter("att", [4, L, BS], F32, isOutput=True)

    edge_d = nc.dram_tensor("edge_d", [NSTEP, H, BS], F32R)

    def mm(out, lhsT, rhs, start, stop):
        nc.tensor.matmul(out, lhsT, rhs, start=start, stop=stop)

    ghv_v = ghvT.rearrange("t (k p) b -> t k p b", p=128)
    edge_v = edge_d.rearrange("t (m p) b -> t m p b", p=128)
    eix_v = eidxT.rearrange("(k p) b -> k p b", p=128)
    enc_v = encT.rearrange("l (k p) b -> l k p b", p=128)

    with tile.TileContext(nc) as tc:
        pers_cm = tc.tile_pool(name="pers", bufs=1)
        pers = pers_cm.__enter__()
        bi_t = pers.tile([128, KH], F32)
        btot_t = pers.tile([128, NC4], F32)
        v4_t = pers.tile([128, NC4, 4], F32R)
        nc.sync.dma_start(bi_t[:], bi[:])
        nc.sync.dma_start(btot_t[:], btot[:])
        nc.sync.dma_start(v4_t[:], v4s[:])

        # We chunk for m=0, prefetched during phase A1 so the A1->A2
        # boundary only waits on the first edge tile.
        a1pre_cm = tc.tile_pool(name="a1pre", bufs=1)
        a1pre = a1pre_cm.__enter__()
        we_t0 = a1pre.tile([128, KD, 128], F32R)

        # ---- Phase A1: edges.  edgeA (Wh,Wv) and edgeB (Wsh,Wsv) into
        # two psum groups, predicated select on (t==0), spill to DRAM.
        with (
            tc.tile_pool(name="a1w", bufs=1) as wp,
            tc.tile_pool(name="a1hv", bufs=2) as hvp,
            tc.tile_pool(name="a1o", bufs=2) as op,
            tc.tile_pool(name="a1ps", bufs=2, space="PSUM") as pp,
        ):
            wt = wp.tile([128, KH, 4, KD, 128], F32R)
            # interleave the first step's inputs with the first weight
            # chunks: the very first matmul group only needs h0 + w4[0,0]
            h0 = hvp.tile([128, KD, BS], F32R, tag="h")
            v0 = hvp.tile([128, KD, BS], F32R, tag="v")
            for k in range(KD):
                nc.sync.dma_start(h0[:, k, :], ghv_v[0, k])
            nc.sync.dma_start(wt[:, 0, 0], w4[0, :, 0])
            nc.sync.dma_start(wt[:, 0, 1], w4[0, :, 1])
            for k in range(KD):
                nc.sync.dma_start(v0[:, k, :], ghv_v[0, KD + k])
            nc.sync.dma_start(wt[:, 0, 2], w4[0, :, 2])
            nc.sync.dma_start(wt[:, 0, 3], w4[0, :, 3])
            for m in range(1, KH):
                for w in range(4):
                    nc.sync.dma_start(wt[:, m, w], w4[m, :, w])
            cmt = wp.tile([128, NSTEP, BS], U8)
            for t in range(NSTEP):
                nc.sync.dma_start(cmt[:, t, :], cmask[t])
            nc.sync.dma_start(we_t0[:], we4[0])
            ht, vt = h0, v0
            for t in range(NSTEP):
                if t > 0:
                    ht = hvp.tile([128, KD, BS], F32R, tag="h")
                    vt = hvp.tile([128, KD, BS], F32R, tag="v")
                    for k in range(KD):
                        nc.sync.dma_start(ht[:, k, :], ghv_v[t, k])
                    for k in range(KD):
                        nc.sync.dma_start(vt[:, k, :], ghv_v[t, KD + k])
                for m in range(KH):
                    pa = pp.tile([128, BS], F32, tag="pa")
                    for k in range(KD):
                        mm(pa[:], wt[:, m, 0, k, :], ht[:, k, :],
                           start=(k == 0), stop=False)
                    for k in range(KD):
                        mm(pa[:], wt[:, m, 1, k, :], vt[:, k, :],
                           start=False, stop=(k == KD - 1))
                    pb = pp.tile([128, BS], F32, tag="pb")
                    for k in range(KD):
                        mm(pb[:], wt[:, m, 2, k, :], ht[:, k, :],
                           start=(k == 0), stop=False)
                    for k in range(KD):
                        mm(pb[:], wt[:, m, 3, k, :], vt[:, k, :],
                           start=False, stop=(k == KD - 1))
                    ot = op.tile([128, BS], F32R, tag="ot")
                    nc.vector.tensor_copy(ot.bitcast(F32)[:], pb[:])
                    nc.vector.copy_predicated(ot.bitcast(F32)[:],
                                              cmt[:, t, :], pa[:])
                    nc.sync.dma_start(edge_v[t, m], ot[:])

        # inp spans through phase B (the tanh input shift)
        spanB_cm = tc.tile_pool(name="spanB", bufs=1)
        spanB = spanB_cm.__enter__()
        inps = spanB.tile([128, KH, BS], F32)

        # ---- Phase A2 (merged): cand_t = edge_t @ We.T;
        #   qt = relu(edge4 + relu(max_t cand_t));   lin = enc_idx @ Wi.T;
        #   q2 = relu(relu(qt + lin + bi) + lin + bi);  inp = q2 @ Wq.T
        with (
            tc.tile_pool(name="a2w", bufs=1) as wp,
            tc.tile_pool(name="a2e", bufs=2) as ep,
            tc.tile_pool(name="a2s", bufs=1) as sp,
            tc.tile_pool(name="a2t", bufs=1) as tp,
            tc.tile_pool(name="a2ps", bufs=2, space="PSUM") as pp,
        ):
            wt = wp.tile([128, KH, KD, 128], F32R)
            wit = wp.tile([128, KH, KD, 128], F32R)
            wqt = wp.tile([128, KH, KD, 128], F32R)
            eix = sp.tile([128, KD, BS], F32R)
            q2 = sp.tile([128, KH, BS], F32R)
            stq = sp.tile([128, KH, BS], F32)  # cand-max, then qt in place
            for m in range(1, KH):
                nc.sync.dma_start(wt[:, m], we4[m])
            for k in range(KD):
                nc.sync.dma_start(eix[:, k, :], eix_v[k])
            for m in range(KH):
                nc.sync.dma_start(wit[:, m], wi4[m])
            for m in range(KH):
                nc.sync.dma_start(wqt[:, m], wq4[m])
            for t in range(NSTEP):
                est = ep.tile([128, KH, BS], F32R, tag="est")
                for m in range(KH):
                    nc.sync.dma_start(est[:, m, :], edge_v[t, m])
                for m in range(KH):
                    ps = pp.tile([128, BS], F32, tag="pc")
                    wsl = we_t0[:] if m == 0 else wt[:, m]
                    for k in range(KD):
                        mm(ps[:], wsl[:, k, :], est[:, k, :],
                           start=(k == 0), stop=(k == KD - 1))
                    if t == 0:
                        nc.vector.tensor_copy(stq[:, m, :], ps[:])
                    else:
                        nc.vector.tensor_max(stq[:, m, :], stq[:, m, :],
                                             ps[:])
                    if t == NSTEP - 1:
                        nc.vector.tensor_relu(stq[:, m, :], stq[:, m, :])
                        nc.vector.tensor_add(
                            stq[:, m, :], est.bitcast(F32)[:, m, :],
                            stq[:, m, :])
                        nc.vector.tensor_relu(stq[:, m, :], stq[:, m, :])
            for m in range(KH):
                ps = pp.tile([128, BS], F32, tag="pl")
                for k in range(KD):
                    mm(ps[:], wit[:, m, k, :], eix[:, k, :],
                       start=(k == 0), stop=(k == KD - 1))
                lin = tp.tile([128, BS], F32, tag="lin")
                nc.vector.tensor_copy(lin[:], ps[:])
                t1 = tp.tile([128, BS], F32, tag="t1")
                nc.vector.tensor_add(t1[:], stq[:, m, :], lin[:])
                q1 = tp.tile([128, BS], F32, tag="q1")
                nc.scalar.activation(q1[:], t1[:], AF.Relu,
                                     bias=bi_t[:, m:m + 1])
                t2 = tp.tile([128, BS], F32, tag="t2")
                nc.vector.tensor_add(t2[:], q1[:], lin[:])
                nc.scalar.activation(q2[:, m, :], t2[:], AF.Relu,
                                     bias=bi_t[:, m:m + 1])
            for m in range(KH):
                ps = pp.tile([128, BS], F32, tag="pq")
                for k in range(KD):
                    mm(ps[:], wqt[:, m, k, :], q2[:, k, :],
                       start=(k == 0), stop=(k == KD - 1))
                nc.vector.tensor_copy(inps[:, m, :], ps[:])

        # ---- Phase B: attention.  For each l:
        #   ctx chunk = Wc-proj; y = tanh(ctx + inp + bias);
        #   att[k, b] += V4seg.T @ y  (accumulated over the 32 chunks)
        with (
            tc.tile_pool(name="bw", bufs=1) as wp,
            tc.tile_pool(name="be", bufs=2) as ep,
            tc.tile_pool(name="bt", bufs=3) as tp,
            tc.tile_pool(name="bps", bufs=3, space="PSUM") as pp,
            tc.tile_pool(name="baps", bufs=2, space="PSUM") as app,
        ):
            et0 = ep.tile([128, KD, BS], F32R, tag="et")
            for k in range(KD):
                nc.sync.dma_start(et0[:, k, :], enc_v[0, k])
            wct = wp.tile([128, NC4, KD, 128], F32R)
            for c4 in range(NC4):
                nc.sync.dma_start(wct[:, c4], wc4[c4])
            et = et0
            for l in range(L):
                if l > 0:
                    et = ep.tile([128, KD, BS], F32R, tag="et")
                    for k in range(KD):
                        nc.sync.dma_start(et[:, k, :], enc_v[l, k])
                attps = app.tile([4, BS], F32, tag="attps")
                ys = []
                for c4 in range(NC4):
                    pc = pp.tile([128, BS], F32, tag="pctx")
                    for k in range(KD):
                        mm(pc[:], wct[:, c4, k, :], et[:, k, :],
                           start=(k == 0), stop=(k == KD - 1))
                    # att matmul for the previous chunk goes after this
                    # group so the PE never waits on DVE/ACT latency.
                    if ys:
                        c4p, yp = ys[-1]
                        mm(attps[:], v4_t[:, c4p, :], yp[:],
                           start=(c4p == 0), stop=False)
                    ypre = tp.tile([128, BS], F32, tag="ypre")
                    nc.vector.tensor_add(ypre[:], pc[:],
                                         inps[:, c4 % KH, :])
                    y = tp.tile([128, BS], F32R, tag="y")
                    nc.scalar.activation(y[:], ypre[:], AF.Tanh,
                                         bias=btot_t[:, c4:c4 + 1])
                    ys.append((c4, y))
                c4p, yp = ys[-1]
                mm(attps[:], v4_t[:, c4p, :], yp[:], start=False, stop=True)
                asb = tp.tile([4, BS], F32, tag="asb")
                nc.vector.tensor_copy(asb[:], attps[:])
                nc.sync.dma_start(att[:, l, :], asb[:])

        spanB_cm.__exit__(None, None, None)
        a1pre_cm.__exit__(None, None, None)
        pers_cm.__exit__(None, None, None)

    nc.finalize()
    return nc


def _get_program():
    global _PROG
    if _PROG is None:
        _PROG = _build_program()
    return _PROG


def _prep_inputs(encoder_output, xes, idx):
    """Build the 8 per-core input maps (all float32 numpy)."""
    enc = np.ascontiguousarray(np.asarray(encoder_output, dtype=np.float32))
    xes = np.asarray(xes)
    idx = np.asarray(idx)
    ar = np.arange(B)

    # [NCORES, L, D, BS]
    encT = np.ascontiguousarray(
        enc.reshape(NCORES, BS, L, D).transpose(0, 2, 3, 1))

    h = enc[ar[:, None], xes[:, :NSTEP, 0]]  # [B, 5, D]
    v = enc[ar[:, None], xes[:, :NSTEP, 1]]
    ghv = np.stack([h, v], axis=2)  # [B, 5, 2, D]
    ghvT = np.ascontiguousarray(
        ghv.reshape(NCORES, BS, NSTEP, 2 * D).transpose(0, 2, 3, 1))

    eidx = enc[ar, idx]  # [B, D]
    eidxT = np.ascontiguousarray(
        eidx.reshape(NCORES, BS, D).transpose(0, 2, 1))

    c = (xes[:, :NSTEP, 2] == 0).astype(np.uint8)  # [B, 5]
    cT = c.reshape(NCORES, BS, NSTEP).transpose(0, 2, 1)  # [NCORES, 5, BS]
    cmask = np.ascontiguousarray(
        np.broadcast_to(cT[:, :, None, :], (NCORES, NSTEP, 128, BS)))

    return encT, ghvT, eidxT, cmask


def _prep_weights(Wq, bq, Wc, bc, V4, Wi, bi, Wh, Wv, Wsh, Wsv, We):
    f = lambda a: np.ascontiguousarray(np.asarray(a, dtype=np.float32))

    def pack(Wt):
        # W.T [d, h] -> [m, p, k, q]  (m = h chunk, k = d chunk)
        return np.ascontiguousarray(
            Wt.reshape(KD, 128, KH, 128).transpose(2, 1, 0, 3))

    w4 = np.stack([pack(f(w).T) for w in (Wh, Wv, Wsh, Wsv)],
                  axis=2)  # [m, p, 4, k, q]
    w4 = np.ascontiguousarray(w4)
    we4 = pack(f(We).T)
    wi4 = pack(f(Wi).T)
    wq4 = pack(f(Wq).T)
    wcT = np.ascontiguousarray(f(Wc).transpose(2, 0, 1).reshape(D, 4 * H))
    wc4 = np.ascontiguousarray(
        wcT.reshape(KD, 128, NC4, 128).transpose(2, 1, 0, 3))  # [c4,p,k,q]
    bi_t = np.ascontiguousarray(f(bi).reshape(KH, 128).T)
    bcq = (f(bc) + f(bq)[None, :]).reshape(4 * H)  # bias for tanh input
    btot = np.ascontiguousarray(bcq.reshape(NC4, 128).T)
    v4s = np.zeros((128, NC4, 4), np.float32)
    V4f = f(V4)
    for c4 in range(NC4):
        k = c4 // KH
        v4s[:, c4, k] = V4f[k, (c4 % KH) * 128:(c4 % KH + 1) * 128]
    return dict(w4=w4, we4=we4, wi4=wi4, wq4=wq4, wc4=wc4,
                bi=bi_t, btot=btot, v4s=v4s)


def run_device(encoder_output, xes, idx, weights, trace=False, trace_cores=None):
    """Run the Bass kernel on 8 cores; returns (att_raw [B, 4, L], results)."""
    from concourse.bass_utils import run_bass_kernel_spmd

    nc = _get_program()
    encT, ghvT, eidxT, cmask = _prep_inputs(encoder_output, xes, idx)
    wmap = _prep_weights(**weights)
    in_maps = []
    for c in range(NCORES):
        m = {"encT": encT[c], "ghvT": ghvT[c], "eidxT": eidxT[c],
             "cmask": cmask[c]}
        m.update(wmap)
        in_maps.append(m)
    res = run_bass_kernel_spmd(nc, in_maps, list(range(NCORES)),
                               trace=trace, trace_cores=trace_cores)
    att = np.stack([r["att"] for r in res.results])  # [8, 4, L, BS]
    att_raw = np.ascontiguousarray(
        att.transpose(0, 3, 1, 2).reshape(B, 4, L))
    return att_raw, res


def _edge_chain_host(enc, xes, idx, W, rows, dtype=np.float64):
    """Exact recompute of att_raw for the given batch rows (vectorized)."""
    f = lambda a: np.asarray(a, dtype=dtype)
    e = f(enc[rows])  # [n, L, D]
    x = np.asarray(xes)[rows]  # [n, T, 3]
    n = len(rows)
    an = np.arange(n)
    Wh, Wv, Wsh, Wsv, We = f(W["Wh"]), f(W["Wv"]), f(W["Wsh"]), f(W["Wsv"]), f(W["We"])
    Wi, Wq, Wc = f(W["Wi"]), f(W["Wq"]), f(W["Wc"])
    bi, bq, bc, V4 = f(W["bi"]), f(W["bq"]), f(W["bc"]), f(W["V4"])

    el = np.zeros((n, H), dtype)
    st = np.zeros((n, H), dtype)
    qt = None
    for t in range(T):
        h = e[an, x[:, t, 0]]
        v = e[an, x[:, t, 1]]
        cond = (x[:, t, 2] == 0)[:, None]
        edge = np.where(cond, h @ Wh.T + v @ Wv.T, v @ Wsv.T + h @ Wsh.T)
        subtree = np.maximum(st, edge @ We.T)
        qt = np.maximum(el + st, 0.0)
        el, st = edge, subtree
    enc_idx = e[an, np.asarray(idx)[rows]]
    lin = enc_idx @ Wi.T + bi
    q = np.maximum(qt + lin, 0.0)
    q = np.maximum(q + lin, 0.0)
    inp = q @ Wq.T + bq
    ctx = np.einsum("nld,khd->knhl", e, Wc) + bc[:, None, :, None]
    y = np.tanh(inp[None, :, :, None] + ctx)
    att_raw = np.einsum("kh,knhl->nkl", V4, y)  # [n, 4, L]
    return att_raw


def kernel(encoder_output, xes, idx, mask, Wq, bq, Wc, bc, V4, Wi, bi,
           Wh, Wv, Wsh, Wsv, We):
    import jax
    import jax.numpy as jnp

    enc = np.asarray(encoder_output, dtype=np.float32)
    xes = np.asarray(xes)
    idx = np.asarray(idx)
    mask = np.asarray(mask)
    weights = dict(Wq=Wq, bq=bq, Wc=Wc, bc=bc, V4=V4, Wi=Wi, bi=bi,
                   Wh=Wh, Wv=Wv, Wsh=Wsh, Wsv=Wsv, We=We)

    att_raw, _ = run_device(enc, xes, idx, weights)  # [B, 4, L]

    def finish(att_raw_f64):
        a = att_raw_f64.reshape(B, 4 * L)
        mask4 = np.tile(mask != 0, (1, 4))
        a = np.where(mask4, a, -np.inf)
        a = 10.0 * np.tanh(a)
        amax = a.max(axis=0)
        ex = np.exp(a - amax[None, :])
        s = ex.sum(axis=0)
        alpha = ex / s[None, :]
        lse = amax + np.log(s)
        logits = a - lse[None, :]
        return a, alpha, logits

    att64 = att_raw.astype(np.float64)
    a, alpha, logits = finish(att64)

    # gumbel noise — exactly what jax.random.categorical(key, logits,
    # axis=1) adds before its argmax
    G = np.asarray(jax.random.gumbel(jax.random.key(42), (B, 4 * L),
                                     jnp.float32), dtype=np.float64)
    pert = logits + G
    part = np.partition(pert, 4 * L - 2, axis=1)
    margin = part[:, -1] - part[:, -2]
    # rescue rows where the argmax could flip under the device's matmul
    # error, and rows whose selected probability is off the tanh
    # saturation plateau (where p inherits the raw att error)
    chosen0 = np.argmax(pert, axis=1)
    raw_sel = np.take_along_axis(att64.reshape(B, 4 * L), chosen0[:, None],
                                 axis=1)[:, 0]
    risky = np.nonzero((margin < DELTA) | (np.abs(raw_sel) < ASAT))[0]
    if len(risky) > 0:
        att64[risky] = _edge_chain_host(enc, xes, idx, weights, risky)
        a, alpha, logits = finish(att64)
        pert = logits + G

    indices = np.argmax(pert, axis=1).astype(np.int32)[:, None]
    p = np.take_along_axis(alpha, indices, axis=1).astype(np.float32)
    one_hot = (np.arange(L)[None, :] == indices).astype(mask.dtype)
    mask_out = mask - one_hot
    return indices, p, mask_out


# revision 26
# speedup vs baseline: 1.0830x; 1.0830x over previous
"""Trainium2 Bass kernel for nn_Decoder_57432302682540.

Strategy (pure data-parallel over batch, 8 NeuronCores):
  - Host: shard B=4096 into 8x512, pre-gather the recurrence rows
    (h_t, v_t, enc[idx]) and pre-transpose everything into [d, b]
    layout so the PE contracts over partitions.
  - Device (per core, all matmuls in fp32r = full-rate reduced-precision
    fp32): 5-step edge computation (both branches + predicated select),
    subtree max-chain via We, query construction (Wi, Wq), then the
    4-way tanh attention (Wc projection, +inp, tanh, V4-weighted
    reduction over h done as a [128,4] matmul) producing raw
    att[k, l, b] per core.
  - Host: mask, 10*tanh, softmax over the batch axis (the cross-shard
    coupling), categorical sampling with jax key 42 (gumbel-argmax,
    identical to jax.random.categorical), p gather and mask update.
    Rows whose sampling margin is within DELTA of a tie are recomputed
    exactly on the host (float64) so reduced-precision matmuls cannot
    flip an argmax.
"""

import numpy as np

B, L, D, H, T = 4096, 7, 1024, 1024, 6
NCORES = 8
BS = B // NCORES  # 512 rows per core
NSTEP = 5  # last scan step's edge/subtree never reaches the output
KD = D // 128  # 8 contraction chunks
KH = H // 128  # 8 output chunks
NC4 = 4 * H // 128  # 32 attention output chunks
JC = 96  # compaction slots per (core, step); overflow rows host-rescued
DELTA = 0.30  # sampling-margin below which rows are recomputed on host
ASAT = 3.0  # |att_raw| at the chosen column below which p is recomputed

_PROG = None  # cached compiled Bass program


def _build_program():
    import concourse.bacc as bacc
    import concourse.mybir as mybir
    from concourse import tile

    F32 = mybir.dt.float32
    F32R = mybir.dt.float32r
    U8 = mybir.dt.uint8
    AF = mybir.ActivationFunctionType

    nc = bacc.Bacc()

    def inp(name, shape, dt=F32):
        return nc.declare_dram_parameter(name, shape, dt, isOutput=False)

    # weights are host-prepacked in consumption order: leading dim is the
    # output chunk the matmul loop consumes, so the first matmul group only
    # waits for its own chunk's DMA.
    encT = inp("encT", [L, D, BS], F32R)
    ghvT = inp("ghvT", [NSTEP, 2 * D, BS], F32R)  # per step: h then v chunks
    eidxT = inp("eidxT", [D, BS], F32R)
    hAT = inp("hAT", [NSTEP, 2, D, JC], F32R)  # compacted t==0 rows, [d, j]
    smat = inp("smat", [NSTEP, JC, BS], F32R)  # scatter matrix [j, b]
    wS = inp("wS", [KH, 128, 2, KD, 128], F32R)  # [m,p,(Wsh,Wsv),k,q]
    wD = inp("wD", [2, KD, 128, H], F32R)  # (Wh-Wsh).T, (Wv-Wsv).T
    we4 = inp("we4", [KH, 128, KD, 128], F32R)  # [m,p,k,q] of We.T
    wi4 = inp("wi4", [KH, 128, KD, 128], F32R)
    wq4 = inp("wq4", [KH, 128, KD, 128], F32R)
    wc4 = inp("wc4", [NC4, 128, KD, 128], F32R)  # [c4,p,k,q] of Wc flat
    bi = inp("bi", [128, KH])
    btot = inp("btot", [128, NC4])  # bc (k-major) + bq, per chunk column
    v4s = inp("v4s", [128, NC4, 4], F32R)  # V4 chunk in column k of its group
    att = nc.declare_dram_parameter("att", [4, L, BS], F32, isOutput=True)

    edge_d = nc.dram_tensor("edge_d", [NSTEP, H, BS], F32R)

    def mm(out, lhsT, rhs, start, stop):
        nc.tensor.matmul(out, lhsT, rhs, start=start, stop=stop)

    ghv_v = ghvT.rearrange("t (k p) b -> t k p b", p=128)
    hA_v = hAT.rearrange("t w (k p) j -> t w k p j", p=128)
    edge_v = edge_d.rearrange("t (m p) b -> t m p b", p=128)
    eix_v = eidxT.rearrange("(k p) b -> k p b", p=128)
    enc_v = encT.rearrange("l (k p) b -> l k p b", p=128)

    with tile.TileContext(nc) as tc:
        # ---- Phase A1: edges.  edge = h@Wsh.T + v@Wsv.T for all rows, plus
        # a compacted correction h@(Wh-Wsh).T + v@(Wv-Wsv).T for the <=128
        # rows per step with t==0, scattered back into the psum group via a
        # 0/1 selection-matrix matmul.  corrT for step t+1 is computed (in
        # [j, h] orientation) during step t's main loop.
        with (
            tc.tile_pool(name="a1w", bufs=1) as wp,
            tc.tile_pool(name="a1hv", bufs=2) as hvp,
            tc.tile_pool(name="a1x", bufs=1) as xp,
            tc.tile_pool(name="a1ps", bufs=2, space="PSUM") as pp,
            tc.tile_pool(name="a1cps", bufs=2, space="PSUM") as cpp,
        ):
            wt = wp.tile([128, KH, 2, KD, 128], F32R)
            hvs, sts, has = {}, {}, {}

            def prefetch(t):
                if t >= NSTEP:
                    return
                ht = hvp.tile([128, KD, BS], F32R, tag="h")
                vt = hvp.tile([128, KD, BS], F32R, tag="v")
                for k in range(KD):
                    nc.sync.dma_start(ht[:, k, :], ghv_v[t, k])
                for k in range(KD):
                    nc.sync.dma_start(vt[:, k, :], ghv_v[t, KD + k])
                st = xp.tile([JC, BS], F32R, tag="smat")
                nc.sync.dma_start(st[:], smat[t])
                ha = xp.tile([128, 2, KD, JC], F32R, tag="hA")
                for w in range(2):
                    for k in range(KD):
                        nc.sync.dma_start(ha[:, w, k], hA_v[t, w, k])
                hvs[t], sts[t], has[t] = (ht, vt), st, ha

            prefetch(0)
            nc.sync.dma_start(wt[:, 0, 0], wS[0, :, 0])
            nc.sync.dma_start(wt[:, 0, 1], wS[0, :, 1])
            wdt = wp.tile([128, 2, KD, H], F32R)
            for w in range(2):
                for k in range(KD):
                    nc.sync.dma_start(wdt[:, w, k, :], wD[w, k])
            for m in range(1, KH):
                for w in range(2):
                    nc.sync.dma_start(wt[:, m, w], wS[m, :, w])

            def corr_mms(t, ct):
                # corrT[j, h] for step t into 2 psum banks -> SBUF tile ct
                ha = has[t]
                for half in range(2):
                    cps = cpp.tile([JC, 512], F32, tag="cps")
                    for w in range(2):
                        for k in range(KD):
                            mm(cps[:], ha[:, w, k],
                               wdt[:, w, k, half * 512:(half + 1) * 512],
                               start=(w == 0 and k == 0),
                               stop=(w == 1 and k == KD - 1))
                    nc.vector.tensor_copy(
                        ct[:, half * 512:(half + 1) * 512], cps[:])

            ct = xp.tile([JC, H], F32R, tag="corrT")
            corr_mms(0, ct)
            for t in range(NSTEP):
                ht, vt = hvs[t]
                st = sts[t]
                prefetch(t + 1)
                ctn = None
                for m in range(KH):
                    pb = pp.tile([128, BS], F32, tag="pb")
                    for k in range(KD):
                        mm(pb[:], wt[:, m, 0, k, :], ht[:, k, :],
                           start=(k == 0), stop=False)
                    for k in range(KD):
                        mm(pb[:], wt[:, m, 1, k, :], vt[:, k, :],
                           start=False, stop=False)
                    mm(pb[:], ct[:, m * 128:(m + 1) * 128], st[:],
                       start=False, stop=True)
                    if m == 3 and t < NSTEP - 1:
                        ctn = xp.tile([JC, H], F32R, tag="corrT")
                        corr_mms(t + 1, ctn)
                    ot = xp.tile([128, BS], F32R, tag="ot")
                    nc.vector.tensor_copy(ot.bitcast(F32)[:], pb[:])
                    nc.sync.dma_start(edge_v[t, m], ot[:])
                if ctn is not None:
                    ct = ctn

        # inp spans through phase B (the tanh input shift)
        spanB_cm = tc.tile_pool(name="spanB", bufs=1)
        spanB = spanB_cm.__enter__()
        inps = spanB.tile([128, KH, BS], F32)

        # ---- Phase A2 (merged): cand_t = edge_t @ We.T;
        #   qt = relu(edge4 + relu(max_t cand_t));   lin = enc_idx @ Wi.T;
        #   q2 = relu(relu(qt + lin + bi) + lin + bi);  inp = q2 @ Wq.T
        with (
            tc.tile_pool(name="a2w", bufs=1) as wp,
            tc.tile_pool(name="a2e", bufs=2) as ep,
            tc.tile_pool(name="a2s", bufs=1) as sp,
            tc.tile_pool(name="a2t", bufs=1) as tp,
            tc.tile_pool(name="a2ps", bufs=2, space="PSUM") as pp,
        ):
            wt = wp.tile([128, KH, KD, 128], F32R)
            wit = wp.tile([128, KH, KD, 128], F32R)
            wqt = wp.tile([128, KH, KD, 128], F32R)
            eix = sp.tile([128, KD, BS], F32R)
            bi_t = sp.tile([128, KH], F32)
            nc.sync.dma_start(bi_t[:], bi[:])
            q2 = sp.tile([128, KH, BS], F32R)
            stq = sp.tile([128, KH, BS], F32)  # cand-max, then qt in place
            ests = []
            est = ep.tile([128, KH, BS], F32R, tag="est")
            for m in range(KH):
                nc.sync.dma_start(est[:, m, :], edge_v[0, m])
            ests.append(est)
            for m in range(KH):
                nc.sync.dma_start(wt[:, m], we4[m])
            est = ep.tile([128, KH, BS], F32R, tag="est")
            for m in range(KH):
                nc.sync.dma_start(est[:, m, :], edge_v[1, m])
            ests.append(est)
            for k in range(KD):
                nc.sync.dma_start(eix[:, k, :], eix_v[k])
            for m in range(KH):
                nc.sync.dma_start(wit[:, m], wi4[m])
            for m in range(KH):
                nc.sync.dma_start(wqt[:, m], wq4[m])
            for t in range(NSTEP):
                if t >= 2:
                    est = ep.tile([128, KH, BS], F32R, tag="est")
                    for m in range(KH):
                        nc.sync.dma_start(est[:, m, :], edge_v[t, m])
                    ests.append(est)
                est = ests[t]
                for m in range(KH):
                    ps = pp.tile([128, BS], F32, tag="pc")
                    for k in range(KD):
                        mm(ps[:], wt[:, m, k, :], est[:, k, :],
                           start=(k == 0), stop=(k == KD - 1))
                    if t == 0:
                        nc.vector.tensor_copy(stq[:, m, :], ps[:])
                    else:
                        nc.vector.tensor_max(stq[:, m, :], stq[:, m, :],
                                             ps[:])
                    if t == NSTEP - 1:
                        nc.vector.tensor_relu(stq[:, m, :], stq[:, m, :])
                        nc.vector.tensor_add(
                            stq[:, m, :], est.bitcast(F32)[:, m, :],
                            stq[:, m, :])
                        nc.vector.tensor_relu(stq[:, m, :], stq[:, m, :])
            for m in range(KH):
                ps = pp.tile([128, BS], F32, tag="pl")
                for k in range(KD):
                    mm(ps[:], wit[:, m, k, :], eix[:, k, :],
                       start=(k == 0), stop=(k == KD - 1))
                lin = tp.tile([128, BS], F32, tag="lin")
                nc.vector.tensor_copy(lin[:], ps[:])
                t1 = tp.tile([128, BS], F32, tag="t1")
                nc.vector.tensor_add(t1[:], stq[:, m, :], lin[:])
                q1 = tp.tile([128, BS], F32, tag="q1")
                nc.scalar.activation(q1[:], t1[:], AF.Relu,
                                     bias=bi_t[:, m:m + 1])
                t2 = tp.tile([128, BS], F32, tag="t2")
                nc.vector.tensor_add(t2[:], q1[:], lin[:])
                nc.scalar.activation(q2[:, m, :], t2[:], AF.Relu,
                                     bias=bi_t[:, m:m + 1])
            for m in range(KH):
                ps = pp.tile([128, BS], F32, tag="pq")
                for k in range(KD):
                    mm(ps[:], wqt[:, m, k, :], q2[:, k, :],
                       start=(k == 0), stop=(k == KD - 1))
                nc.vector.tensor_copy(inps[:, m, :], ps[:])

        # ---- Phase B: attention.  For each l:
        #   ctx chunk = Wc-proj; y = tanh(ctx + inp + bias);
        #   att[k, b] += V4seg.T @ y  (accumulated over the 32 chunks)
        with (
            tc.tile_pool(name="bw", bufs=1) as wp,
            tc.tile_pool(name="be", bufs=2) as ep,
            tc.tile_pool(name="bt", bufs=3) as tp,
            tc.tile_pool(name="bps", bufs=3, space="PSUM") as pp,
            tc.tile_pool(name="baps", bufs=2, space="PSUM") as app,
        ):
            btot_t = wp.tile([128, NC4], F32)
            v4_t = wp.tile([128, NC4, 4], F32R)
            nc.sync.dma_start(btot_t[:], btot[:])
            nc.sync.dma_start(v4_t[:], v4s[:])
            et0 = ep.tile([128, KD, BS], F32R, tag="et")
            for k in range(KD):
                nc.sync.dma_start(et0[:, k, :], enc_v[0, k])
            wct = wp.tile([128, NC4, KD, 128], F32R)
            for c4 in range(NC4):
                nc.sync.dma_start(wct[:, c4], wc4[c4])
            et = et0
            for l in range(L):
                if l > 0:
                    et = ep.tile([128, KD, BS], F32R, tag="et")
                    for k in range(KD):
                        nc.sync.dma_start(et[:, k, :], enc_v[l, k])
                attps = app.tile([4, BS], F32, tag="attps")
                ys = []
                for c4 in range(NC4):
                    pc = pp.tile([128, BS], F32, tag="pctx")
                    for k in range(KD):
                        mm(pc[:], wct[:, c4, k, :], et[:, k, :],
                           start=(k == 0), stop=(k == KD - 1))
                    # att matmul for the previous chunk goes after this
                    # group so the PE never waits on DVE/ACT latency.
                    if ys:
                        c4p, yp = ys[-1]
                        mm(attps[:], v4_t[:, c4p, :], yp[:],
                           start=(c4p == 0), stop=False)
                    ypre = tp.tile([128, BS], F32, tag="ypre")
                    nc.vector.tensor_add(ypre[:], pc[:],
                                         inps[:, c4 % KH, :])
                    y = tp.tile([128, BS], F32R, tag="y")
                    nc.scalar.activation(y[:], ypre[:], AF.Tanh,
                                         bias=btot_t[:, c4:c4 + 1])
                    ys.append((c4, y))
                c4p, yp = ys[-1]
                mm(attps[:], v4_t[:, c4p, :], yp[:], start=False, stop=True)
                asb = tp.tile([4, BS], F32, tag="asb")
                nc.vector.tensor_copy(asb[:], attps[:])
                nc.sync.dma_start(att[:, l, :], asb[:])

        spanB_cm.__exit__(None, None, None)

    nc.finalize()
    return nc


def _get_program():
    global _PROG
    if _PROG is None:
        _PROG = _build_program()
    return _PROG


def _prep_inputs(encoder_output, xes, idx):
    """Build the 8 per-core input maps (all float32 numpy)."""
    enc = np.ascontiguousarray(np.asarray(encoder_output, dtype=np.float32))
    xes = np.asarray(xes)
    idx = np.asarray(idx)
    ar = np.arange(B)

    # [NCORES, L, D, BS]
    encT = np.ascontiguousarray(
        enc.reshape(NCORES, BS, L, D).transpose(0, 2, 3, 1))

    h = enc[ar[:, None], xes[:, :NSTEP, 0]]  # [B, 5, D]
    v = enc[ar[:, None], xes[:, :NSTEP, 1]]
    ghv = np.stack([h, v], axis=2)  # [B, 5, 2, D]
    ghvT = np.ascontiguousarray(
        ghv.reshape(NCORES, BS, NSTEP, 2 * D).transpose(0, 2, 3, 1))

    eidx = enc[ar, idx]  # [B, D]
    eidxT = np.ascontiguousarray(
        eidx.reshape(NCORES, BS, D).transpose(0, 2, 1))

    # compacted branch-A (t==0) rows per (core, step) + scatter matrix
    cond = (xes[:, :NSTEP, 2] == 0)  # [B, 5]
    hAT = np.zeros((NCORES, NSTEP, 2, D, JC), np.float32)
    smat = np.zeros((NCORES, NSTEP, JC, BS), np.float32)
    overflow = []
    for c in range(NCORES):
        for t in range(NSTEP):
            rows = np.nonzero(cond[c * BS:(c + 1) * BS, t])[0]
            if len(rows) > JC:
                overflow.extend((c * BS + rows[JC:]).tolist())
                rows = rows[:JC]
            n = len(rows)
            grows = c * BS + rows
            hAT[c, t, 0, :, :n] = h[grows, t].T
            hAT[c, t, 1, :, :n] = v[grows, t].T
            smat[c, t, np.arange(n), rows] = 1.0
    return encT, ghvT, eidxT, hAT, smat, np.array(overflow, np.int64)


def _prep_weights(Wq, bq, Wc, bc, V4, Wi, bi, Wh, Wv, Wsh, Wsv, We):
    f = lambda a: np.ascontiguousarray(np.asarray(a, dtype=np.float32))

    def pack(Wt):
        # W.T [d, h] -> [m, p, k, q]  (m = h chunk, k = d chunk)
        return np.ascontiguousarray(
            Wt.reshape(KD, 128, KH, 128).transpose(2, 1, 0, 3))

    wS = np.ascontiguousarray(np.stack(
        [pack(f(Wsh).T), pack(f(Wsv).T)], axis=2))  # [m, p, 2, k, q]
    wD = np.ascontiguousarray(np.stack([
        (f(Wh) - f(Wsh)).T.reshape(KD, 128, H),
        (f(Wv) - f(Wsv)).T.reshape(KD, 128, H)]))  # [2, k, p, h]
    we4 = pack(f(We).T)
    wi4 = pack(f(Wi).T)
    wq4 = pack(f(Wq).T)
    wcT = np.ascontiguousarray(f(Wc).transpose(2, 0, 1).reshape(D, 4 * H))
    wc4 = np.ascontiguousarray(
        wcT.reshape(KD, 128, NC4, 128).transpose(2, 1, 0, 3))  # [c4,p,k,q]
    bi_t = np.ascontiguousarray(f(bi).reshape(KH, 128).T)
    bcq = (f(bc) + f(bq)[None, :]).reshape(4 * H)  # bias for tanh input
    btot = np.ascontiguousarray(bcq.reshape(NC4, 128).T)
    v4s = np.zeros((128, NC4, 4), np.float32)
    V4f = f(V4)
    for c4 in range(NC4):
        k = c4 // KH
        v4s[:, c4, k] = V4f[k, (c4 % KH) * 128:(c4 % KH + 1) * 128]
    return dict(wS=wS, wD=wD, we4=we4, wi4=wi4, wq4=wq4, wc4=wc4,
                bi=bi_t, btot=btot, v4s=v4s)


def run_device(encoder_output, xes, idx, weights, trace=False,
               trace_cores=None):
    """Run the Bass kernel on 8 cores; returns (att_raw [B,4,L], overflow, res)."""
    from concourse.bass_utils import run_bass_kernel_spmd

    nc = _get_program()
    encT, ghvT, eidxT, hAT, smat, overflow = _prep_inputs(
        encoder_output, xes, idx)
    wmap = _prep_weights(**weights)
    in_maps = []
    for c in range(NCORES):
        m = {"encT": encT[c], "ghvT": ghvT[c], "eidxT": eidxT[c],
             "hAT": hAT[c], "smat": smat[c]}
        m.update(wmap)
        in_maps.append(m)
    res = run_bass_kernel_spmd(nc, in_maps, list(range(NCORES)),
                               trace=trace, trace_cores=trace_cores)
    att = np.stack([r["att"] for r in res.results])  # [8, 4, L, BS]
    att_raw = np.ascontiguousarray(
        att.transpose(0, 3, 1, 2).reshape(B, 4, L))
    return att_raw, overflow, res


def _edge_chain_host(enc, xes, idx, W, rows, dtype=np.float64):
    """Exact recompute of att_raw for the given batch rows (vectorized)."""
    f = lambda a: np.asarray(a, dtype=dtype)
    e = f(enc[rows])  # [n, L, D]
    x = np.asarray(xes)[rows]  # [n, T, 3]
    n = len(rows)
    an = np.arange(n)
    Wh, Wv, Wsh, Wsv, We = f(W["Wh"]), f(W["Wv"]), f(W["Wsh"]), f(W["Wsv"]), f(W["We"])
    Wi, Wq, Wc = f(W["Wi"]), f(W["Wq"]), f(W["Wc"])
    bi, bq, bc, V4 = f(W["bi"]), f(W["bq"]), f(W["bc"]), f(W["V4"])

    el = np.zeros((n, H), dtype)
    st = np.zeros((n, H), dtype)
    qt = None
    for t in range(T):
        h = e[an, x[:, t, 0]]
        v = e[an, x[:, t, 1]]
        cond = (x[:, t, 2] == 0)[:, None]
        edge = np.where(cond, h @ Wh.T + v @ Wv.T, v @ Wsv.T + h @ Wsh.T)
        subtree = np.maximum(st, edge @ We.T)
        qt = np.maximum(el + st, 0.0)
        el, st = edge, subtree
    enc_idx = e[an, np.asarray(idx)[rows]]
    lin = enc_idx @ Wi.T + bi
    q = np.maximum(qt + lin, 0.0)
    q = np.maximum(q + lin, 0.0)
    inp = q @ Wq.T + bq
    ctx = np.einsum("nld,khd->knhl", e, Wc) + bc[:, None, :, None]
    y = np.tanh(inp[None, :, :, None] + ctx)
    att_raw = np.einsum("kh,knhl->nkl", V4, y)  # [n, 4, L]
    return att_raw


def kernel(encoder_output, xes, idx, mask, Wq, bq, Wc, bc, V4, Wi, bi,
           Wh, Wv, Wsh, Wsv, We):
    import jax
    import jax.numpy as jnp

    enc = np.asarray(encoder_output, dtype=np.float32)
    xes = np.asarray(xes)
    idx = np.asarray(idx)
    mask = np.asarray(mask)
    weights = dict(Wq=Wq, bq=bq, Wc=Wc, bc=bc, V4=V4, Wi=Wi, bi=bi,
                   Wh=Wh, Wv=Wv, Wsh=Wsh, Wsv=Wsv, We=We)

    att_raw, overflow, _ = run_device(enc, xes, idx, weights)  # [B, 4, L]

    def finish(att_raw_f64):
        a = att_raw_f64.reshape(B, 4 * L)
        mask4 = np.tile(mask != 0, (1, 4))
        a = np.where(mask4, a, -np.inf)
        a = 10.0 * np.tanh(a)
        amax = a.max(axis=0)
        ex = np.exp(a - amax[None, :])
        s = ex.sum(axis=0)
        alpha = ex / s[None, :]
        lse = amax + np.log(s)
        logits = a - lse[None, :]
        return a, alpha, logits

    att64 = att_raw.astype(np.float64)
    a, alpha, logits = finish(att64)

    # gumbel noise — exactly what jax.random.categorical(key, logits,
    # axis=1) adds before its argmax
    G = np.asarray(jax.random.gumbel(jax.random.key(42), (B, 4 * L),
                                     jnp.float32), dtype=np.float64)
    pert = logits + G
    part = np.partition(pert, 4 * L - 2, axis=1)
    margin = part[:, -1] - part[:, -2]
    # rescue rows where the argmax could flip under the device's matmul
    # error, and rows whose selected probability is off the tanh
    # saturation plateau (where p inherits the raw att error)
    chosen0 = np.argmax(pert, axis=1)
    raw_sel = np.take_along_axis(att64.reshape(B, 4 * L), chosen0[:, None],
                                 axis=1)[:, 0]
    riskmask = (margin < DELTA) | (np.abs(raw_sel) < ASAT)
    if len(overflow):
        riskmask[overflow] = True
    risky = np.nonzero(riskmask)[0]
    if len(risky) > 0:
        att64[risky] = _edge_chain_host(enc, xes, idx, weights, risky)
        a, alpha, logits = finish(att64)
        pert = logits + G

    indices = np.argmax(pert, axis=1).astype(np.int32)[:, None]
    p = np.take_along_axis(alpha, indices, axis=1).astype(np.float32)
    one_hot = (np.arange(L)[None, :] == indices).astype(mask.dtype)
    mask_out = mask - one_hot
    return indices, p, mask_out


# revision 27
# speedup vs baseline: 1.0843x; 1.0012x over previous
"""Trainium2 Bass kernel for nn_Decoder_57432302682540.

Strategy (pure data-parallel over batch, 8 NeuronCores):
  - Host: shard B=4096 into 8x512, pre-gather the recurrence rows
    (h_t, v_t, enc[idx]) and pre-transpose everything into [d, b]
    layout so the PE contracts over partitions.
  - Device (per core, all matmuls in fp32r = full-rate reduced-precision
    fp32): 5-step edge computation (both branches + predicated select),
    subtree max-chain via We, query construction (Wi, Wq), then the
    4-way tanh attention (Wc projection, +inp, tanh, V4-weighted
    reduction over h done as a [128,4] matmul) producing raw
    att[k, l, b] per core.
  - Host: mask, 10*tanh, softmax over the batch axis (the cross-shard
    coupling), categorical sampling with jax key 42 (gumbel-argmax,
    identical to jax.random.categorical), p gather and mask update.
    Rows whose sampling margin is within DELTA of a tie are recomputed
    exactly on the host (float64) so reduced-precision matmuls cannot
    flip an argmax.
"""

import numpy as np

B, L, D, H, T = 4096, 7, 1024, 1024, 6
NCORES = 8
BS = B // NCORES  # 512 rows per core
NSTEP = 5  # last scan step's edge/subtree never reaches the output
KD = D // 128  # 8 contraction chunks
KH = H // 128  # 8 output chunks
NC4 = 4 * H // 128  # 32 attention output chunks
JC = 96  # compaction slots per (core, step); overflow rows host-rescued
DELTA = 0.30  # sampling-margin below which rows are recomputed on host
ASAT = 3.0  # |att_raw| at the chosen column below which p is recomputed

_PROG = None  # cached compiled Bass program


def _build_program():
    import concourse.bacc as bacc
    import concourse.mybir as mybir
    from concourse import tile

    F32 = mybir.dt.float32
    F32R = mybir.dt.float32r
    U8 = mybir.dt.uint8
    AF = mybir.ActivationFunctionType

    nc = bacc.Bacc()

    def inp(name, shape, dt=F32):
        return nc.declare_dram_parameter(name, shape, dt, isOutput=False)

    # weights are host-prepacked in consumption order: leading dim is the
    # output chunk the matmul loop consumes, so the first matmul group only
    # waits for its own chunk's DMA.
    encT = inp("encT", [L, D, BS], F32R)
    ghvT = inp("ghvT", [NSTEP, 2 * D, BS], F32R)  # per step: h then v chunks
    eidxT = inp("eidxT", [D, BS], F32R)
    hAT = inp("hAT", [NSTEP, 2, D, JC], F32R)  # compacted t==0 rows, [d, j]
    smat = inp("smat", [NSTEP, JC, BS], F32R)  # scatter matrix [j, b]
    wS = inp("wS", [KH, 128, 2, KD, 128], F32R)  # [m,p,(Wsh,Wsv),k,q]
    wD = inp("wD", [2, KD, 128, H], F32R)  # (Wh-Wsh).T, (Wv-Wsv).T
    we4 = inp("we4", [KH, 128, KD, 128], F32R)  # [m,p,k,q] of We.T
    wi4 = inp("wi4", [KH, 128, KD, 128], F32R)
    wq4 = inp("wq4", [KH, 128, KD, 128], F32R)
    wc4 = inp("wc4", [NC4, 128, KD, 128], F32R)  # [c4,p,k,q] of Wc flat
    bi = inp("bi", [128, KH])
    btot = inp("btot", [128, NC4])  # bc (k-major) + bq, per chunk column
    v4s = inp("v4s", [128, NC4, 4], F32R)  # V4 chunk in column k of its group
    att = nc.declare_dram_parameter("att", [4, L, BS], F32, isOutput=True)

    edge_d = nc.dram_tensor("edge_d", [NSTEP, H, BS], F32R)

    def mm(out, lhsT, rhs, start, stop):
        nc.tensor.matmul(out, lhsT, rhs, start=start, stop=stop)

    ghv_v = ghvT.rearrange("t (k p) b -> t k p b", p=128)
    hA_v = hAT.rearrange("t w (k p) j -> t w k p j", p=128)
    edge_v = edge_d.rearrange("t (m p) b -> t m p b", p=128)
    eix_v = eidxT.rearrange("(k p) b -> k p b", p=128)
    enc_v = encT.rearrange("l (k p) b -> l k p b", p=128)

    with tile.TileContext(nc) as tc:
        # ---- Phase A1: edges.  edge = h@Wsh.T + v@Wsv.T for all rows, plus
        # a compacted correction h@(Wh-Wsh).T + v@(Wv-Wsv).T for the <=128
        # rows per step with t==0, scattered back into the psum group via a
        # 0/1 selection-matrix matmul.  corrT for step t+1 is computed (in
        # [j, h] orientation) during step t's main loop.
        with (
            tc.tile_pool(name="a1w", bufs=1) as wp,
            tc.tile_pool(name="a1hv", bufs=2) as hvp,
            tc.tile_pool(name="a1x", bufs=1) as xp,
            tc.tile_pool(name="a1ps", bufs=2, space="PSUM") as pp,
            tc.tile_pool(name="a1cps", bufs=2, space="PSUM") as cpp,
        ):
            wt = wp.tile([128, KH, 2, KD, 128], F32R)
            hvs, sts, has = {}, {}, {}

            def prefetch(t):
                if t >= NSTEP:
                    return
                ht = hvp.tile([128, KD, BS], F32R, tag="h")
                vt = hvp.tile([128, KD, BS], F32R, tag="v")
                for k in range(KD):
                    nc.sync.dma_start(ht[:, k, :], ghv_v[t, k])
                for k in range(KD):
                    nc.sync.dma_start(vt[:, k, :], ghv_v[t, KD + k])
                st = xp.tile([JC, BS], F32R, tag="smat")
                nc.sync.dma_start(st[:], smat[t])
                ha = xp.tile([128, 2, KD, JC], F32R, tag="hA")
                for w in range(2):
                    for k in range(KD):
                        nc.sync.dma_start(ha[:, w, k], hA_v[t, w, k])
                hvs[t], sts[t], has[t] = (ht, vt), st, ha

            prefetch(0)
            nc.sync.dma_start(wt[:, 0, 0], wS[0, :, 0])
            nc.sync.dma_start(wt[:, 0, 1], wS[0, :, 1])
            nc.sync.dma_start(wt[:, 1, 0], wS[1, :, 0])
            nc.sync.dma_start(wt[:, 1, 1], wS[1, :, 1])
            wdt = wp.tile([128, 2, KD, H], F32R)
            for w in range(2):
                for k in range(KD):
                    nc.sync.dma_start(wdt[:, w, k, :], wD[w, k])
            for m in range(2, KH):
                for w in range(2):
                    nc.sync.dma_start(wt[:, m, w], wS[m, :, w])

            def corr_mms(t, ct):
                # corrT[j, h] for step t into 2 psum banks -> SBUF tile ct
                ha = has[t]
                for half in range(2):
                    cps = cpp.tile([JC, 512], F32, tag="cps")
                    for w in range(2):
                        for k in range(KD):
                            mm(cps[:], ha[:, w, k],
                               wdt[:, w, k, half * 512:(half + 1) * 512],
                               start=(w == 0 and k == 0),
                               stop=(w == 1 and k == KD - 1))
                    nc.vector.tensor_copy(
                        ct[:, half * 512:(half + 1) * 512], cps[:])

            ct = None
            for t in range(NSTEP):
                ht, vt = hvs[t]
                st = sts[t]
                prefetch(t + 1)
                ctn = None
                pend = []
                for m in range(KH):
                    pb = pp.tile([128, BS], F32, tag="pb")
                    for k in range(KD):
                        mm(pb[:], wt[:, m, 0, k, :], ht[:, k, :],
                           start=(k == 0), stop=False)
                    for k in range(KD):
                        mm(pb[:], wt[:, m, 1, k, :], vt[:, k, :],
                           start=False, stop=False)
                    if t == 0 and m < 2:
                        # defer the scatter so the PE isn't gated on the
                        # (large) wD prologue DMA for its first groups
                        pend.append((m, pb))
                        if m == 1:
                            ct = xp.tile([JC, H], F32R, tag="corrT")
                            corr_mms(0, ct)
                            for mp, pbp in pend:
                                mm(pbp[:],
                                   ct[:, mp * 128:(mp + 1) * 128], st[:],
                                   start=False, stop=True)
                                ot = xp.tile([128, BS], F32R, tag="ot")
                                nc.vector.tensor_copy(
                                    ot.bitcast(F32)[:], pbp[:])
                                nc.sync.dma_start(edge_v[t, mp], ot[:])
                        continue
                    mm(pb[:], ct[:, m * 128:(m + 1) * 128], st[:],
                       start=False, stop=True)
                    if m == 3 and t < NSTEP - 1:
                        ctn = xp.tile([JC, H], F32R, tag="corrT")
                        corr_mms(t + 1, ctn)
                    ot = xp.tile([128, BS], F32R, tag="ot")
                    nc.vector.tensor_copy(ot.bitcast(F32)[:], pb[:])
                    nc.sync.dma_start(edge_v[t, m], ot[:])
                if ctn is not None:
                    ct = ctn

        # inp spans through phase B (the tanh input shift)
        spanB_cm = tc.tile_pool(name="spanB", bufs=1)
        spanB = spanB_cm.__enter__()
        inps = spanB.tile([128, KH, BS], F32)

        # ---- Phase A2 (merged): cand_t = edge_t @ We.T;
        #   qt = relu(edge4 + relu(max_t cand_t));   lin = enc_idx @ Wi.T;
        #   q2 = relu(relu(qt + lin + bi) + lin + bi);  inp = q2 @ Wq.T
        with (
            tc.tile_pool(name="a2w", bufs=1) as wp,
            tc.tile_pool(name="a2e", bufs=2) as ep,
            tc.tile_pool(name="a2s", bufs=1) as sp,
            tc.tile_pool(name="a2t", bufs=1) as tp,
            tc.tile_pool(name="a2ps", bufs=2, space="PSUM") as pp,
        ):
            wt = wp.tile([128, KH, KD, 128], F32R)
            wit = wp.tile([128, KH, KD, 128], F32R)
            wqt = wp.tile([128, KH, KD, 128], F32R)
            eix = sp.tile([128, KD, BS], F32R)
            bi_t = sp.tile([128, KH], F32)
            nc.sync.dma_start(bi_t[:], bi[:])
            q2 = sp.tile([128, KH, BS], F32R)
            stq = sp.tile([128, KH, BS], F32)  # cand-max, then qt in place
            ests = []

            def est_load(t):
                e = ep.tile([128, KH, BS], F32R, tag="est")
                for m in range(KH):
                    nc.sync.dma_start(e[:, m, :], edge_v[t, m])
                ests.append(e)

            est_load(0)
            for m in range(KH):
                nc.sync.dma_start(wt[:, m], we4[m])
            est_load(1)
            for k in range(KD):
                nc.sync.dma_start(eix[:, k, :], eix_v[k])
            est_load(2)
            for m in range(KH):
                nc.sync.dma_start(wit[:, m], wi4[m])
            est_load(3)
            for m in range(KH):
                nc.sync.dma_start(wqt[:, m], wq4[m])
            est_load(4)
            for t in range(NSTEP):
                est = ests[t]
                for m in range(KH):
                    ps = pp.tile([128, BS], F32, tag="pc")
                    for k in range(KD):
                        mm(ps[:], wt[:, m, k, :], est[:, k, :],
                           start=(k == 0), stop=(k == KD - 1))
                    if t == 0:
                        nc.vector.tensor_copy(stq[:, m, :], ps[:])
                    else:
                        nc.vector.tensor_max(stq[:, m, :], stq[:, m, :],
                                             ps[:])
                    if t == NSTEP - 1:
                        nc.vector.tensor_relu(stq[:, m, :], stq[:, m, :])
                        nc.vector.tensor_add(
                            stq[:, m, :], est.bitcast(F32)[:, m, :],
                            stq[:, m, :])
                        nc.vector.tensor_relu(stq[:, m, :], stq[:, m, :])
            for m in range(KH):
                ps = pp.tile([128, BS], F32, tag="pl")
                for k in range(KD):
                    mm(ps[:], wit[:, m, k, :], eix[:, k, :],
                       start=(k == 0), stop=(k == KD - 1))
                lin = tp.tile([128, BS], F32, tag="lin")
                nc.vector.tensor_copy(lin[:], ps[:])
                t1 = tp.tile([128, BS], F32, tag="t1")
                nc.vector.tensor_add(t1[:], stq[:, m, :], lin[:])
                q1 = tp.tile([128, BS], F32, tag="q1")
                nc.scalar.activation(q1[:], t1[:], AF.Relu,
                                     bias=bi_t[:, m:m + 1])
                t2 = tp.tile([128, BS], F32, tag="t2")
                nc.vector.tensor_add(t2[:], q1[:], lin[:])
                nc.scalar.activation(q2[:, m, :], t2[:], AF.Relu,
                                     bias=bi_t[:, m:m + 1])
            for m in range(KH):
                ps = pp.tile([128, BS], F32, tag="pq")
                for k in range(KD):
                    mm(ps[:], wqt[:, m, k, :], q2[:, k, :],
                       start=(k == 0), stop=(k == KD - 1))
                nc.vector.tensor_copy(inps[:, m, :], ps[:])

        # ---- Phase B: attention.  For each l:
        #   ctx chunk = Wc-proj; y = tanh(ctx + inp + bias);
        #   att[k, b] += V4seg.T @ y  (accumulated over the 32 chunks)
        with (
            tc.tile_pool(name="bw", bufs=1) as wp,
            tc.tile_pool(name="be", bufs=2) as ep,
            tc.tile_pool(name="bt", bufs=3) as tp,
            tc.tile_pool(name="bps", bufs=3, space="PSUM") as pp,
            tc.tile_pool(name="baps", bufs=2, space="PSUM") as app,
        ):
            btot_t = wp.tile([128, NC4], F32)
            v4_t = wp.tile([128, NC4, 4], F32R)
            nc.sync.dma_start(btot_t[:], btot[:])
            nc.sync.dma_start(v4_t[:], v4s[:])
            et0 = ep.tile([128, KD, BS], F32R, tag="et")
            for k in range(KD):
                nc.sync.dma_start(et0[:, k, :], enc_v[0, k])
            wct = wp.tile([128, NC4, KD, 128], F32R)
            for c4 in range(NC4):
                nc.sync.dma_start(wct[:, c4], wc4[c4])
            et = et0
            for l in range(L):
                if l > 0:
                    et = ep.tile([128, KD, BS], F32R, tag="et")
                    for k in range(KD):
                        nc.sync.dma_start(et[:, k, :], enc_v[l, k])
                attps = app.tile([4, BS], F32, tag="attps")
                ys = []
                for c4 in range(NC4):
                    pc = pp.tile([128, BS], F32, tag="pctx")
                    for k in range(KD):
                        mm(pc[:], wct[:, c4, k, :], et[:, k, :],
                           start=(k == 0), stop=(k == KD - 1))
                    # att matmul for the previous chunk goes after this
                    # group so the PE never waits on DVE/ACT latency.
                    if ys:
                        c4p, yp = ys[-1]
                        mm(attps[:], v4_t[:, c4p, :], yp[:],
                           start=(c4p == 0), stop=False)
                    ypre = tp.tile([128, BS], F32, tag="ypre")
                    nc.vector.tensor_add(ypre[:], pc[:],
                                         inps[:, c4 % KH, :])
                    y = tp.tile([128, BS], F32R, tag="y")
                    nc.scalar.activation(y[:], ypre[:], AF.Tanh,
                                         bias=btot_t[:, c4:c4 + 1])
                    ys.append((c4, y))
                c4p, yp = ys[-1]
                mm(attps[:], v4_t[:, c4p, :], yp[:], start=False, stop=True)
                asb = tp.tile([4, BS], F32, tag="asb")
                nc.vector.tensor_copy(asb[:], attps[:])
                nc.sync.dma_start(att[:, l, :], asb[:])

        spanB_cm.__exit__(None, None, None)

    nc.finalize()
    return nc


def _get_program():
    global _PROG
    if _PROG is None:
        _PROG = _build_program()
    return _PROG


def _prep_inputs(encoder_output, xes, idx):
    """Build the 8 per-core input maps (all float32 numpy)."""
    enc = np.ascontiguousarray(np.asarray(encoder_output, dtype=np.float32))
    xes = np.asarray(xes)
    idx = np.asarray(idx)
    ar = np.arange(B)

    # [NCORES, L, D, BS]
    encT = np.ascontiguousarray(
        enc.reshape(NCORES, BS, L, D).transpose(0, 2, 3, 1))

    h = enc[ar[:, None], xes[:, :NSTEP, 0]]  # [B, 5, D]
    v = enc[ar[:, None], xes[:, :NSTEP, 1]]
    ghv = np.stack([h, v], axis=2)  # [B, 5, 2, D]
    ghvT = np.ascontiguousarray(
        ghv.reshape(NCORES, BS, NSTEP, 2 * D).transpose(0, 2, 3, 1))

    eidx = enc[ar, idx]  # [B, D]
    eidxT = np.ascontiguousarray(
        eidx.reshape(NCORES, BS, D).transpose(0, 2, 1))

    # compacted branch-A (t==0) rows per (core, step) + scatter matrix
    cond = (xes[:, :NSTEP, 2] == 0)  # [B, 5]
    hAT = np.zeros((NCORES, NSTEP, 2, D, JC), np.float32)
    smat = np.zeros((NCORES, NSTEP, JC, BS), np.float32)
    overflow = []
    for c in range(NCORES):
        for t in range(NSTEP):
            rows = np.nonzero(cond[c * BS:(c + 1) * BS, t])[0]
            if len(rows) > JC:
                overflow.extend((c * BS + rows[JC:]).tolist())
                rows = rows[:JC]
            n = len(rows)
            grows = c * BS + rows
            hAT[c, t, 0, :, :n] = h[grows, t].T
            hAT[c, t, 1, :, :n] = v[grows, t].T
            smat[c, t, np.arange(n), rows] = 1.0
    return encT, ghvT, eidxT, hAT, smat, np.array(overflow, np.int64)


def _prep_weights(Wq, bq, Wc, bc, V4, Wi, bi, Wh, Wv, Wsh, Wsv, We):
    f = lambda a: np.ascontiguousarray(np.asarray(a, dtype=np.float32))

    def pack(Wt):
        # W.T [d, h] -> [m, p, k, q]  (m = h chunk, k = d chunk)
        return np.ascontiguousarray(
            Wt.reshape(KD, 128, KH, 128).transpose(2, 1, 0, 3))

    wS = np.ascontiguousarray(np.stack(
        [pack(f(Wsh).T), pack(f(Wsv).T)], axis=2))  # [m, p, 2, k, q]
    wD = np.ascontiguousarray(np.stack([
        (f(Wh) - f(Wsh)).T.reshape(KD, 128, H),
        (f(Wv) - f(Wsv)).T.reshape(KD, 128, H)]))  # [2, k, p, h]
    we4 = pack(f(We).T)
    wi4 = pack(f(Wi).T)
    wq4 = pack(f(Wq).T)
    wcT = np.ascontiguousarray(f(Wc).transpose(2, 0, 1).reshape(D, 4 * H))
    wc4 = np.ascontiguousarray(
        wcT.reshape(KD, 128, NC4, 128).transpose(2, 1, 0, 3))  # [c4,p,k,q]
    bi_t = np.ascontiguousarray(f(bi).reshape(KH, 128).T)
    bcq = (f(bc) + f(bq)[None, :]).reshape(4 * H)  # bias for tanh input
    btot = np.ascontiguousarray(bcq.reshape(NC4, 128).T)
    v4s = np.zeros((128, NC4, 4), np.float32)
    V4f = f(V4)
    for c4 in range(NC4):
        k = c4 // KH
        v4s[:, c4, k] = V4f[k, (c4 % KH) * 128:(c4 % KH + 1) * 128]
    return dict(wS=wS, wD=wD, we4=we4, wi4=wi4, wq4=wq4, wc4=wc4,
                bi=bi_t, btot=btot, v4s=v4s)


def run_device(encoder_output, xes, idx, weights, trace=False,
               trace_cores=None):
    """Run the Bass kernel on 8 cores; returns (att_raw [B,4,L], overflow, res)."""
    from concourse.bass_utils import run_bass_kernel_spmd

    nc = _get_program()
    encT, ghvT, eidxT, hAT, smat, overflow = _prep_inputs(
        encoder_output, xes, idx)
    wmap = _prep_weights(**weights)
    in_maps = []
    for c in range(NCORES):
        m = {"encT": encT[c], "ghvT": ghvT[c], "eidxT": eidxT[c],
             "hAT": hAT[c], "smat": smat[c]}
        m.update(wmap)
        in_maps.append(m)
    res = run_bass_kernel_spmd(nc, in_maps, list(range(NCORES)),
                               trace=trace, trace_cores=trace_cores)
    att = np.stack([r["att"] for r in res.results])  # [8, 4, L, BS]
    att_raw = np.ascontiguousarray(
        att.transpose(0, 3, 1, 2).reshape(B, 4, L))
    return att_raw, overflow, res


def _edge_chain_host(enc, xes, idx, W, rows, dtype=np.float64):
    """Exact recompute of att_raw for the given batch rows (vectorized)."""
    f = lambda a: np.asarray(a, dtype=dtype)
    e = f(enc[rows])  # [n, L, D]
    x = np.asarray(xes)[rows]  # [n, T, 3]
    n = len(rows)
    an = np.arange(n)
    Wh, Wv, Wsh, Wsv, We = f(W["Wh"]), f(W["Wv"]), f(W["Wsh"]), f(W["Wsv"]), f(W["We"])
    Wi, Wq, Wc = f(W["Wi"]), f(W["Wq"]), f(W["Wc"])
    bi, bq, bc, V4 = f(W["bi"]), f(W["bq"]), f(W["bc"]), f(W["V4"])

    el = np.zeros((n, H), dtype)
    st = np.zeros((n, H), dtype)
    qt = None
    for t in range(T):
        h = e[an, x[:, t, 0]]
        v = e[an, x[:, t, 1]]
        cond = (x[:, t, 2] == 0)[:, None]
        edge = np.where(cond, h @ Wh.T + v @ Wv.T, v @ Wsv.T + h @ Wsh.T)
        subtree = np.maximum(st, edge @ We.T)
        qt = np.maximum(el + st, 0.0)
        el, st = edge, subtree
    enc_idx = e[an, np.asarray(idx)[rows]]
    lin = enc_idx @ Wi.T + bi
    q = np.maximum(qt + lin, 0.0)
    q = np.maximum(q + lin, 0.0)
    inp = q @ Wq.T + bq
    ctx = np.einsum("nld,khd->knhl", e, Wc) + bc[:, None, :, None]
    y = np.tanh(inp[None, :, :, None] + ctx)
    att_raw = np.einsum("kh,knhl->nkl", V4, y)  # [n, 4, L]
    return att_raw


def kernel(encoder_output, xes, idx, mask, Wq, bq, Wc, bc, V4, Wi, bi,
           Wh, Wv, Wsh, Wsv, We):
    import jax
    import jax.numpy as jnp

    enc = np.asarray(encoder_output, dtype=np.float32)
    xes = np.asarray(xes)
    idx = np.asarray(idx)
    mask = np.asarray(mask)
    weights = dict(Wq=Wq, bq=bq, Wc=Wc, bc=bc, V4=V4, Wi=Wi, bi=bi,
                   Wh=Wh, Wv=Wv, Wsh=Wsh, Wsv=Wsv, We=We)

    att_raw, overflow, _ = run_device(enc, xes, idx, weights)  # [B, 4, L]

    def finish(att_raw_f64):
        a = att_raw_f64.reshape(B, 4 * L)
        mask4 = np.tile(mask != 0, (1, 4))
        a = np.where(mask4, a, -np.inf)
        a = 10.0 * np.tanh(a)
        amax = a.max(axis=0)
        ex = np.exp(a - amax[None, :])
        s = ex.sum(axis=0)
        alpha = ex / s[None, :]
        lse = amax + np.log(s)
        logits = a - lse[None, :]
        return a, alpha, logits

    att64 = att_raw.astype(np.float64)
    a, alpha, logits = finish(att64)

    # gumbel noise — exactly what jax.random.categorical(key, logits,
    # axis=1) adds before its argmax
    G = np.asarray(jax.random.gumbel(jax.random.key(42), (B, 4 * L),
                                     jnp.float32), dtype=np.float64)
    pert = logits + G
    part = np.partition(pert, 4 * L - 2, axis=1)
    margin = part[:, -1] - part[:, -2]
    # rescue rows where the argmax could flip under the device's matmul
    # error, and rows whose selected probability is off the tanh
    # saturation plateau (where p inherits the raw att error)
    chosen0 = np.argmax(pert, axis=1)
    raw_sel = np.take_along_axis(att64.reshape(B, 4 * L), chosen0[:, None],
                                 axis=1)[:, 0]
    riskmask = (margin < DELTA) | (np.abs(raw_sel) < ASAT)
    if len(overflow):
        riskmask[overflow] = True
    risky = np.nonzero(riskmask)[0]
    if len(risky) > 0:
        att64[risky] = _edge_chain_host(enc, xes, idx, weights, risky)
        a, alpha, logits = finish(att64)
        pert = logits + G

    indices = np.argmax(pert, axis=1).astype(np.int32)[:, None]
    p = np.take_along_axis(alpha, indices, axis=1).astype(np.float32)
    one_hot = (np.arange(L)[None, :] == indices).astype(mask.dtype)
    mask_out = mask - one_hot
    return indices, p, mask_out


# revision 29
# speedup vs baseline: 1.0925x; 1.0076x over previous
"""Trainium2 Bass kernel for nn_Decoder_57432302682540.

Strategy (pure data-parallel over batch, 8 NeuronCores):
  - Host: shard B=4096 into 8x512, pre-gather the recurrence rows
    (h_t, v_t, enc[idx]) and pre-transpose everything into [d, b]
    layout so the PE contracts over partitions.
  - Device (per core, all matmuls in fp32r = full-rate reduced-precision
    fp32): 5-step edge computation (both branches + predicated select),
    subtree max-chain via We, query construction (Wi, Wq), then the
    4-way tanh attention (Wc projection, +inp, tanh, V4-weighted
    reduction over h done as a [128,4] matmul) producing raw
    att[k, l, b] per core.
  - Host: mask, 10*tanh, softmax over the batch axis (the cross-shard
    coupling), categorical sampling with jax key 42 (gumbel-argmax,
    identical to jax.random.categorical), p gather and mask update.
    Rows whose sampling margin is within DELTA of a tie are recomputed
    exactly on the host (float64) so reduced-precision matmuls cannot
    flip an argmax.
"""

import numpy as np

B, L, D, H, T = 4096, 7, 1024, 1024, 6
NCORES = 8
BS = B // NCORES  # 512 rows per core
NSTEP = 5  # last scan step's edge/subtree never reaches the output
KD = D // 128  # 8 contraction chunks
KH = H // 128  # 8 output chunks
NC4 = 4 * H // 128  # 32 attention output chunks
JC = 96  # compaction slots per (core, step); overflow rows host-rescued
DELTA = 0.30  # sampling-margin below which rows are recomputed on host
ASAT = 3.0  # |att_raw| at the chosen column below which p is recomputed

_PROG = None  # cached compiled Bass program


def _build_program():
    import concourse.bacc as bacc
    import concourse.mybir as mybir
    from concourse import tile

    F32 = mybir.dt.float32
    F32R = mybir.dt.float32r
    U8 = mybir.dt.uint8
    AF = mybir.ActivationFunctionType

    nc = bacc.Bacc()

    def inp(name, shape, dt=F32):
        return nc.declare_dram_parameter(name, shape, dt, isOutput=False)

    # weights are host-prepacked in consumption order: leading dim is the
    # output chunk the matmul loop consumes, so the first matmul group only
    # waits for its own chunk's DMA.
    encT = inp("encT", [L, D, BS], F32R)
    ghvT = inp("ghvT", [NSTEP, 2 * D, BS], F32R)  # per step: h then v chunks
    eidxT = inp("eidxT", [D, BS], F32R)
    hAT = inp("hAT", [NSTEP, 2, D, JC], F32R)  # compacted t==0 rows, [d, j]
    smat = inp("smat", [NSTEP, JC, BS], F32R)  # scatter matrix [j, b]
    wS = inp("wS", [KH, 128, 2, KD, 128], F32R)  # [m,p,(Wsh,Wsv),k,q]
    wD = inp("wD", [2, KD, 128, H], F32R)  # (Wh-Wsh).T, (Wv-Wsv).T
    we4 = inp("we4", [KH, 128, KD, 128], F32R)  # [m,p,k,q] of We.T
    wi4 = inp("wi4", [KH, 128, KD, 128], F32R)
    wq4 = inp("wq4", [KH, 128, KD, 128], F32R)
    wc4 = inp("wc4", [NC4, 128, KD, 128], F32R)  # [c4,p,k,q] of Wc flat
    bi = inp("bi", [128, KH])
    btot = inp("btot", [128, NC4])  # bc (k-major) + bq, per chunk column
    v4s = inp("v4s", [128, NC4, 4], F32R)  # V4 chunk in column k of its group
    att = nc.declare_dram_parameter("att", [4, L, BS], F32, isOutput=True)

    edge_d = nc.dram_tensor("edge_d", [NSTEP, H, BS], F32R)

    def mm(out, lhsT, rhs, start, stop):
        nc.tensor.matmul(out, lhsT, rhs, start=start, stop=stop)

    ghv_v = ghvT.rearrange("t (k p) b -> t k p b", p=128)
    hA_v = hAT.rearrange("t w (k p) j -> t w k p j", p=128)
    edge_v = edge_d.rearrange("t (m p) b -> t m p b", p=128)
    eix_v = eidxT.rearrange("(k p) b -> k p b", p=128)
    enc_v = encT.rearrange("l (k p) b -> l k p b", p=128)

    with tile.TileContext(nc) as tc:
        # ---- Phase A1: edges.  edge = h@Wsh.T + v@Wsv.T for all rows, plus
        # a compacted correction h@(Wh-Wsh).T + v@(Wv-Wsv).T for the <=128
        # rows per step with t==0, scattered back into the psum group via a
        # 0/1 selection-matrix matmul.  corrT for step t+1 is computed (in
        # [j, h] orientation) during step t's main loop.
        with (
            tc.tile_pool(name="a1w", bufs=1) as wp,
            tc.tile_pool(name="a1hv", bufs=2) as hvp,
            tc.tile_pool(name="a1x", bufs=1) as xp,
            tc.tile_pool(name="a1ps", bufs=2, space="PSUM") as pp,
            tc.tile_pool(name="a1cps", bufs=2, space="PSUM") as cpp,
        ):
            wt = wp.tile([128, KH, 2, KD, 128], F32R)
            hvs, sts, has = {}, {}, {}

            def prefetch(t):
                if t >= NSTEP:
                    return
                ht = hvp.tile([128, KD, BS], F32R, tag="h")
                vt = hvp.tile([128, KD, BS], F32R, tag="v")
                for k in range(KD):
                    nc.sync.dma_start(ht[:, k, :], ghv_v[t, k])
                for k in range(KD):
                    nc.sync.dma_start(vt[:, k, :], ghv_v[t, KD + k])
                st = xp.tile([JC, BS], F32R, tag="smat")
                nc.sync.dma_start(st[:], smat[t])
                ha = xp.tile([128, 2, KD, JC], F32R, tag="hA")
                for w in range(2):
                    for k in range(KD):
                        nc.sync.dma_start(ha[:, w, k], hA_v[t, w, k])
                hvs[t], sts[t], has[t] = (ht, vt), st, ha

            # t=0 inputs by hand, interleaved so the first pb group's
            # operands (h0 + wS[0]) land first
            h0 = hvp.tile([128, KD, BS], F32R, tag="h")
            v0 = hvp.tile([128, KD, BS], F32R, tag="v")
            for k in range(KD):
                nc.sync.dma_start(h0[:, k, :], ghv_v[0, k])
            nc.sync.dma_start(wt[:, 0, 0], wS[0, :, 0])
            nc.sync.dma_start(wt[:, 0, 1], wS[0, :, 1])
            for k in range(KD):
                nc.sync.dma_start(v0[:, k, :], ghv_v[0, KD + k])
            nc.sync.dma_start(wt[:, 1, 0], wS[1, :, 0])
            nc.sync.dma_start(wt[:, 1, 1], wS[1, :, 1])
            s0 = xp.tile([JC, BS], F32R, tag="smat")
            nc.sync.dma_start(s0[:], smat[0])
            ha0 = xp.tile([128, 2, KD, JC], F32R, tag="hA")
            for w in range(2):
                for k in range(KD):
                    nc.sync.dma_start(ha0[:, w, k], hA_v[0, w, k])
            hvs[0], sts[0], has[0] = (h0, v0), s0, ha0
            wdt = wp.tile([128, 2, KD, H], F32R)
            for w in range(2):
                for k in range(KD):
                    nc.sync.dma_start(wdt[:, w, k, :], wD[w, k])
            for m in range(2, KH):
                for w in range(2):
                    nc.sync.dma_start(wt[:, m, w], wS[m, :, w])

            def corr_mms(t, ct):
                # corrT[j, h] for step t into 2 psum banks -> SBUF tile ct
                ha = has[t]
                for half in range(2):
                    cps = cpp.tile([JC, 512], F32, tag="cps")
                    for w in range(2):
                        for k in range(KD):
                            mm(cps[:], ha[:, w, k],
                               wdt[:, w, k, half * 512:(half + 1) * 512],
                               start=(w == 0 and k == 0),
                               stop=(w == 1 and k == KD - 1))
                    nc.vector.tensor_copy(
                        ct[:, half * 512:(half + 1) * 512], cps[:])

            ct = None
            for t in range(NSTEP):
                ht, vt = hvs[t]
                st = sts[t]
                prefetch(t + 1)
                ctn = None
                pend = []
                for m in range(KH):
                    pb = pp.tile([128, BS], F32, tag="pb")
                    for k in range(KD):
                        mm(pb[:], wt[:, m, 0, k, :], ht[:, k, :],
                           start=(k == 0), stop=False)
                    for k in range(KD):
                        mm(pb[:], wt[:, m, 1, k, :], vt[:, k, :],
                           start=False, stop=False)
                    if t == 0 and m < 2:
                        # defer the scatter so the PE isn't gated on the
                        # (large) wD prologue DMA for its first groups
                        pend.append((m, pb))
                        if m == 1:
                            ct = xp.tile([JC, H], F32R, tag="corrT")
                            corr_mms(0, ct)
                            for mp, pbp in pend:
                                mm(pbp[:],
                                   ct[:, mp * 128:(mp + 1) * 128], st[:],
                                   start=False, stop=True)
                                ot = xp.tile([128, BS], F32R, tag="ot")
                                nc.vector.tensor_copy(
                                    ot.bitcast(F32)[:], pbp[:])
                                nc.sync.dma_start(edge_v[t, mp], ot[:])
                        continue
                    mm(pb[:], ct[:, m * 128:(m + 1) * 128], st[:],
                       start=False, stop=True)
                    if m == 3 and t < NSTEP - 1:
                        ctn = xp.tile([JC, H], F32R, tag="corrT")
                        corr_mms(t + 1, ctn)
                    ot = xp.tile([128, BS], F32R, tag="ot")
                    nc.vector.tensor_copy(ot.bitcast(F32)[:], pb[:])
                    nc.sync.dma_start(edge_v[t, m], ot[:])
                if ctn is not None:
                    ct = ctn

        # inp spans through phase B (the tanh input shift)
        spanB_cm = tc.tile_pool(name="spanB", bufs=1)
        spanB = spanB_cm.__enter__()
        inps = spanB.tile([128, KH, BS], F32)

        # ---- Phase A2 (merged): cand_t = edge_t @ We.T;
        #   qt = relu(edge4 + relu(max_t cand_t));   lin = enc_idx @ Wi.T;
        #   q2 = relu(relu(qt + lin + bi) + lin + bi);  inp = q2 @ Wq.T
        with (
            tc.tile_pool(name="a2w", bufs=1) as wp,
            tc.tile_pool(name="a2e", bufs=2) as ep,
            tc.tile_pool(name="a2s", bufs=1) as sp,
            tc.tile_pool(name="a2t", bufs=1) as tp,
            tc.tile_pool(name="a2ps", bufs=2, space="PSUM") as pp,
        ):
            wt = wp.tile([128, KH, KD, 128], F32R)
            wit = wp.tile([128, KH, KD, 128], F32R)
            wqt = wp.tile([128, KH, KD, 128], F32R)
            eix = sp.tile([128, KD, BS], F32R)
            bi_t = sp.tile([128, KH], F32)
            nc.sync.dma_start(bi_t[:], bi[:])
            q2 = sp.tile([128, KH, BS], F32R)
            stq = sp.tile([128, KH, BS], F32)  # cand-max, then qt in place
            ests = []

            def est_load(t):
                e = ep.tile([128, KH, BS], F32R, tag="est")
                for m in range(KH):
                    nc.sync.dma_start(e[:, m, :], edge_v[t, m])
                ests.append(e)

            est_load(0)
            for m in range(KH):
                nc.sync.dma_start(wt[:, m], we4[m])
            est_load(1)
            for k in range(KD):
                nc.sync.dma_start(eix[:, k, :], eix_v[k])
            est_load(2)
            for m in range(KH):
                nc.sync.dma_start(wit[:, m], wi4[m])
            est_load(3)
            for m in range(KH):
                nc.sync.dma_start(wqt[:, m], wq4[m])
            est_load(4)
            for t in range(NSTEP):
                est = ests[t]
                for m in range(KH):
                    ps = pp.tile([128, BS], F32, tag="pc")
                    for k in range(KD):
                        mm(ps[:], wt[:, m, k, :], est[:, k, :],
                           start=(k == 0), stop=(k == KD - 1))
                    if t == 0:
                        nc.vector.tensor_copy(stq[:, m, :], ps[:])
                    else:
                        nc.vector.tensor_max(stq[:, m, :], stq[:, m, :],
                                             ps[:])
                    if t == NSTEP - 1:
                        nc.vector.tensor_relu(stq[:, m, :], stq[:, m, :])
                        nc.vector.tensor_add(
                            stq[:, m, :], est.bitcast(F32)[:, m, :],
                            stq[:, m, :])
                        nc.vector.tensor_relu(stq[:, m, :], stq[:, m, :])
            for m in range(KH):
                ps = pp.tile([128, BS], F32, tag="pl")
                for k in range(KD):
                    mm(ps[:], wit[:, m, k, :], eix[:, k, :],
                       start=(k == 0), stop=(k == KD - 1))
                lin = tp.tile([128, BS], F32, tag="lin")
                nc.vector.tensor_copy(lin[:], ps[:])
                t1 = tp.tile([128, BS], F32, tag="t1")
                nc.vector.tensor_add(t1[:], stq[:, m, :], lin[:])
                q1 = tp.tile([128, BS], F32, tag="q1")
                nc.scalar.activation(q1[:], t1[:], AF.Relu,
                                     bias=bi_t[:, m:m + 1])
                t2 = tp.tile([128, BS], F32, tag="t2")
                nc.vector.tensor_add(t2[:], q1[:], lin[:])
                nc.scalar.activation(q2[:, m, :], t2[:], AF.Relu,
                                     bias=bi_t[:, m:m + 1])
            for m in range(KH):
                ps = pp.tile([128, BS], F32, tag="pq")
                for k in range(KD):
                    mm(ps[:], wqt[:, m, k, :], q2[:, k, :],
                       start=(k == 0), stop=(k == KD - 1))
                nc.vector.tensor_copy(inps[:, m, :], ps[:])

        # ---- Phase B: attention.  For each l:
        #   ctx chunk = Wc-proj; y = tanh(ctx + inp + bias);
        #   att[k, b] += V4seg.T @ y  (accumulated over the 32 chunks)
        with (
            tc.tile_pool(name="bw", bufs=1) as wp,
            tc.tile_pool(name="be", bufs=2) as ep,
            tc.tile_pool(name="bt", bufs=3) as tp,
            tc.tile_pool(name="bps", bufs=3, space="PSUM") as pp,
            tc.tile_pool(name="baps", bufs=2, space="PSUM") as app,
        ):
            btot_t = wp.tile([128, NC4], F32)
            v4_t = wp.tile([128, NC4, 4], F32R)
            nc.sync.dma_start(btot_t[:], btot[:])
            nc.sync.dma_start(v4_t[:], v4s[:])
            et0 = ep.tile([128, KD, BS], F32R, tag="et")
            for k in range(KD):
                nc.sync.dma_start(et0[:, k, :], enc_v[0, k])
            wct = wp.tile([128, NC4, KD, 128], F32R)
            for c4 in range(NC4):
                nc.sync.dma_start(wct[:, c4], wc4[c4])
            et = et0
            for l in range(L):
                if l > 0:
                    et = ep.tile([128, KD, BS], F32R, tag="et")
                    for k in range(KD):
                        nc.sync.dma_start(et[:, k, :], enc_v[l, k])
                attps = app.tile([4, BS], F32, tag="attps")
                ys = []
                for c4 in range(NC4):
                    pc = pp.tile([128, BS], F32, tag="pctx")
                    for k in range(KD):
                        mm(pc[:], wct[:, c4, k, :], et[:, k, :],
                           start=(k == 0), stop=(k == KD - 1))
                    # att matmul for the previous chunk goes after this
                    # group so the PE never waits on DVE/ACT latency.
                    if ys:
                        c4p, yp = ys[-1]
                        mm(attps[:], v4_t[:, c4p, :], yp[:],
                           start=(c4p == 0), stop=False)
                    ypre = tp.tile([128, BS], F32, tag="ypre")
                    nc.vector.tensor_add(ypre[:], pc[:],
                                         inps[:, c4 % KH, :])
                    y = tp.tile([128, BS], F32R, tag="y")
                    nc.scalar.activation(y[:], ypre[:], AF.Tanh,
                                         bias=btot_t[:, c4:c4 + 1])
                    ys.append((c4, y))
                c4p, yp = ys[-1]
                mm(attps[:], v4_t[:, c4p, :], yp[:], start=False, stop=True)
                asb = tp.tile([4, BS], F32, tag="asb")
                nc.vector.tensor_copy(asb[:], attps[:])
                nc.sync.dma_start(att[:, l, :], asb[:])

        spanB_cm.__exit__(None, None, None)

    nc.finalize()
    return nc


def _get_program():
    global _PROG
    if _PROG is None:
        _PROG = _build_program()
    return _PROG


def _prep_inputs(encoder_output, xes, idx):
    """Build the 8 per-core input maps (all float32 numpy)."""
    enc = np.ascontiguousarray(np.asarray(encoder_output, dtype=np.float32))
    xes = np.asarray(xes)
    idx = np.asarray(idx)
    ar = np.arange(B)

    # [NCORES, L, D, BS]
    encT = np.ascontiguousarray(
        enc.reshape(NCORES, BS, L, D).transpose(0, 2, 3, 1))

    h = enc[ar[:, None], xes[:, :NSTEP, 0]]  # [B, 5, D]
    v = enc[ar[:, None], xes[:, :NSTEP, 1]]
    ghv = np.stack([h, v], axis=2)  # [B, 5, 2, D]
    ghvT = np.ascontiguousarray(
        ghv.reshape(NCORES, BS, NSTEP, 2 * D).transpose(0, 2, 3, 1))

    eidx = enc[ar, idx]  # [B, D]
    eidxT = np.ascontiguousarray(
        eidx.reshape(NCORES, BS, D).transpose(0, 2, 1))

    # compacted branch-A (t==0) rows per (core, step) + scatter matrix
    cond = (xes[:, :NSTEP, 2] == 0)  # [B, 5]
    hAT = np.zeros((NCORES, NSTEP, 2, D, JC), np.float32)
    smat = np.zeros((NCORES, NSTEP, JC, BS), np.float32)
    overflow = []
    for c in range(NCORES):
        for t in range(NSTEP):
            rows = np.nonzero(cond[c * BS:(c + 1) * BS, t])[0]
            if len(rows) > JC:
                overflow.extend((c * BS + rows[JC:]).tolist())
                rows = rows[:JC]
            n = len(rows)
            grows = c * BS + rows
            hAT[c, t, 0, :, :n] = h[grows, t].T
            hAT[c, t, 1, :, :n] = v[grows, t].T
            smat[c, t, np.arange(n), rows] = 1.0
    return encT, ghvT, eidxT, hAT, smat, np.array(overflow, np.int64)


def _prep_weights(Wq, bq, Wc, bc, V4, Wi, bi, Wh, Wv, Wsh, Wsv, We):
    f = lambda a: np.ascontiguousarray(np.asarray(a, dtype=np.float32))

    def pack(Wt):
        # W.T [d, h] -> [m, p, k, q]  (m = h chunk, k = d chunk)
        return np.ascontiguousarray(
            Wt.reshape(KD, 128, KH, 128).transpose(2, 1, 0, 3))

    wS = np.ascontiguousarray(np.stack(
        [pack(f(Wsh).T), pack(f(Wsv).T)], axis=2))  # [m, p, 2, k, q]
    wD = np.ascontiguousarray(np.stack([
        (f(Wh) - f(Wsh)).T.reshape(KD, 128, H),
        (f(Wv) - f(Wsv)).T.reshape(KD, 128, H)]))  # [2, k, p, h]
    we4 = pack(f(We).T)
    wi4 = pack(f(Wi).T)
    wq4 = pack(f(Wq).T)
    wcT = np.ascontiguousarray(f(Wc).transpose(2, 0, 1).reshape(D, 4 * H))
    wc4 = np.ascontiguousarray(
        wcT.reshape(KD, 128, NC4, 128).transpose(2, 1, 0, 3))  # [c4,p,k,q]
    bi_t = np.ascontiguousarray(f(bi).reshape(KH, 128).T)
    bcq = (f(bc) + f(bq)[None, :]).reshape(4 * H)  # bias for tanh input
    btot = np.ascontiguousarray(bcq.reshape(NC4, 128).T)
    v4s = np.zeros((128, NC4, 4), np.float32)
    V4f = f(V4)
    for c4 in range(NC4):
        k = c4 // KH
        v4s[:, c4, k] = V4f[k, (c4 % KH) * 128:(c4 % KH + 1) * 128]
    return dict(wS=wS, wD=wD, we4=we4, wi4=wi4, wq4=wq4, wc4=wc4,
                bi=bi_t, btot=btot, v4s=v4s)


def run_device(encoder_output, xes, idx, weights, trace=False,
               trace_cores=None):
    """Run the Bass kernel on 8 cores; returns (att_raw [B,4,L], overflow, res)."""
    from concourse.bass_utils import run_bass_kernel_spmd

    nc = _get_program()
    encT, ghvT, eidxT, hAT, smat, overflow = _prep_inputs(
        encoder_output, xes, idx)
    wmap = _prep_weights(**weights)
    in_maps = []
    for c in range(NCORES):
        m = {"encT": encT[c], "ghvT": ghvT[c], "eidxT": eidxT[c],
             "hAT": hAT[c], "smat": smat[c]}
        m.update(wmap)
        in_maps.append(m)
    res = run_bass_kernel_spmd(nc, in_maps, list(range(NCORES)),
                               trace=trace, trace_cores=trace_cores)
    att = np.stack([r["att"] for r in res.results])  # [8, 4, L, BS]
    att_raw = np.ascontiguousarray(
        att.transpose(0, 3, 1, 2).reshape(B, 4, L))
    return att_raw, overflow, res


def _edge_chain_host(enc, xes, idx, W, rows):
    """Exact recompute of att_raw for the given batch rows (BLAS float32)."""
    f = lambda a: np.asarray(a, dtype=np.float32)
    e = f(enc)[rows]  # [n, L, D]
    x = np.asarray(xes)[rows]
    n = len(rows)
    an = np.arange(n)
    WhT, WvT = f(W["Wh"]).T, f(W["Wv"]).T
    WshT, WsvT = f(W["Wsh"]).T, f(W["Wsv"]).T
    WeT, WiT, WqT = f(W["We"]).T, f(W["Wi"]).T, f(W["Wq"]).T
    Wc, bi, bq, bc, V4 = f(W["Wc"]), f(W["bi"]), f(W["bq"]), f(W["bc"]), f(W["V4"])

    el = np.zeros((n, H), np.float32)
    st = np.zeros((n, H), np.float32)
    qt = None
    for t in range(T):
        h = e[an, x[:, t, 0]]
        v = e[an, x[:, t, 1]]
        cond = (x[:, t, 2] == 0)[:, None]
        edge = np.where(cond, h @ WhT + v @ WvT, v @ WsvT + h @ WshT)
        subtree = np.maximum(st, edge @ WeT)
        qt = np.maximum(el + st, 0.0)
        el, st = edge, subtree
    enc_idx = e[an, np.asarray(idx)[rows]]
    lin = enc_idx @ WiT + bi
    q = np.maximum(qt + lin, 0.0)
    q = np.maximum(q + lin, 0.0)
    inp = q @ WqT + bq  # [n, H]
    WcT2 = np.ascontiguousarray(Wc.transpose(2, 0, 1).reshape(D, 4 * H))
    ctx = (e.reshape(n * L, D) @ WcT2).reshape(n, L, 4, H)
    y = np.tanh(inp[:, None, None, :] + ctx + bc[None, None, :, :])
    att_raw = np.einsum("nlkh,kh->nkl", y, V4, optimize=True)  # [n, 4, L]
    return att_raw.astype(np.float64)


def kernel(encoder_output, xes, idx, mask, Wq, bq, Wc, bc, V4, Wi, bi,
           Wh, Wv, Wsh, Wsv, We):
    import jax
    import jax.numpy as jnp

    enc = np.asarray(encoder_output, dtype=np.float32)
    xes = np.asarray(xes)
    idx = np.asarray(idx)
    mask = np.asarray(mask)
    weights = dict(Wq=Wq, bq=bq, Wc=Wc, bc=bc, V4=V4, Wi=Wi, bi=bi,
                   Wh=Wh, Wv=Wv, Wsh=Wsh, Wsv=Wsv, We=We)

    att_raw, overflow, _ = run_device(enc, xes, idx, weights)  # [B, 4, L]

    def finish(att_raw_f64):
        a = att_raw_f64.reshape(B, 4 * L)
        mask4 = np.tile(mask != 0, (1, 4))
        a = np.where(mask4, a, -np.inf)
        a = 10.0 * np.tanh(a)
        amax = a.max(axis=0)
        ex = np.exp(a - amax[None, :])
        s = ex.sum(axis=0)
        alpha = ex / s[None, :]
        lse = amax + np.log(s)
        logits = a - lse[None, :]
        return a, alpha, logits

    att64 = att_raw.astype(np.float64)
    a, alpha, logits = finish(att64)

    # gumbel noise — exactly what jax.random.categorical(key, logits,
    # axis=1) adds before its argmax
    G = np.asarray(jax.random.gumbel(jax.random.key(42), (B, 4 * L),
                                     jnp.float32), dtype=np.float64)
    pert = logits + G
    part = np.partition(pert, 4 * L - 2, axis=1)
    margin = part[:, -1] - part[:, -2]
    # rescue rows where the argmax could flip under the device's matmul
    # error, and rows whose selected probability is off the tanh
    # saturation plateau (where p inherits the raw att error)
    chosen0 = np.argmax(pert, axis=1)
    raw_sel = np.take_along_axis(att64.reshape(B, 4 * L), chosen0[:, None],
                                 axis=1)[:, 0]
    riskmask = (margin < DELTA) | (np.abs(raw_sel) < ASAT)
    if len(overflow):
        riskmask[overflow] = True
    risky = np.nonzero(riskmask)[0]
    if len(risky) > 0:
        att64[risky] = _edge_chain_host(enc, xes, idx, weights, risky)
        a, alpha, logits = finish(att64)
        pert = logits + G

    indices = np.argmax(pert, axis=1).astype(np.int32)[:, None]
    p = np.take_along_axis(alpha, indices, axis=1).astype(np.float32)
    one_hot = (np.arange(L)[None, :] == indices).astype(mask.dtype)
    mask_out = mask - one_hot
    return indices, p, mask_out


# revision 30
# speedup vs baseline: 1.1170x; 1.0224x over previous
"""Trainium2 Bass kernel for nn_Decoder_57432302682540.

Strategy (pure data-parallel over batch, 8 NeuronCores):
  - Host: shard B=4096 into 8x512, pre-gather the recurrence rows
    (h_t, v_t, enc[idx]) and pre-transpose everything into [d, b]
    layout so the PE contracts over partitions.
  - Device (per core, all matmuls in fp32r = full-rate reduced-precision
    fp32): 5-step edge computation (both branches + predicated select),
    subtree max-chain via We, query construction (Wi, Wq), then the
    4-way tanh attention (Wc projection, +inp, tanh, V4-weighted
    reduction over h done as a [128,4] matmul) producing raw
    att[k, l, b] per core.
  - Host: mask, 10*tanh, softmax over the batch axis (the cross-shard
    coupling), categorical sampling with jax key 42 (gumbel-argmax,
    identical to jax.random.categorical), p gather and mask update.
    Rows whose sampling margin is within DELTA of a tie are recomputed
    exactly on the host (float64) so reduced-precision matmuls cannot
    flip an argmax.
"""

import numpy as np

B, L, D, H, T = 4096, 7, 1024, 1024, 6
NCORES = 8
BS = B // NCORES  # 512 rows per core
NSTEP = 5  # last scan step's edge/subtree never reaches the output
KD = D // 128  # 8 contraction chunks
KH = H // 128  # 8 output chunks
NC4 = 4 * H // 128  # 32 attention output chunks
JC = 96  # compaction slots per (core, step); overflow rows host-rescued
DELTA = 0.30  # sampling-margin below which rows are recomputed on host
ASAT = 3.0  # |att_raw| at the chosen column below which p is recomputed

_PROG = None  # cached compiled Bass program


def _build_program():
    import concourse.bacc as bacc
    import concourse.mybir as mybir
    from concourse import tile

    F32 = mybir.dt.float32
    F32R = mybir.dt.float32r
    U8 = mybir.dt.uint8
    AF = mybir.ActivationFunctionType

    nc = bacc.Bacc()

    def inp(name, shape, dt=F32):
        return nc.declare_dram_parameter(name, shape, dt, isOutput=False)

    # weights are host-prepacked in consumption order: leading dim is the
    # output chunk the matmul loop consumes, so the first matmul group only
    # waits for its own chunk's DMA.
    encT = inp("encT", [L, D, BS], F32R)
    ghvT = inp("ghvT", [NSTEP, 2 * D, BS], F32R)  # per step: h then v chunks
    eidxT = inp("eidxT", [D, BS], F32R)
    hAT = inp("hAT", [NSTEP, 2, D, JC], F32R)  # compacted t==0 rows, [d, j]
    smat = inp("smat", [NSTEP, JC, BS], F32R)  # scatter matrix [j, b]
    wS = inp("wS", [KH, 128, 2, KD, 128], F32R)  # [m,p,(Wsh,Wsv),k,q]
    wD = inp("wD", [2, KD, 128, H], F32R)  # (Wh-Wsh).T, (Wv-Wsv).T
    we4 = inp("we4", [KH, 128, KD, 128], F32R)  # [m,p,k,q] of We.T
    wi4 = inp("wi4", [KH, 128, KD, 128], F32R)
    wq4 = inp("wq4", [KH, 128, KD, 128], F32R)
    wc4 = inp("wc4", [NC4, 128, KD, 128], F32R)  # [c4,p,k,q] of Wc flat
    bi = inp("bi", [128, KH])
    btot = inp("btot", [128, NC4])  # bc (k-major) + bq, per chunk column
    v4s = inp("v4s", [128, NC4, 4], F32R)  # V4 chunk in column k of its group
    att = nc.declare_dram_parameter("att", [4, L, BS], F32, isOutput=True)

    edge_d = nc.dram_tensor("edge_d", [NSTEP, H, BS], F32R)

    def mm(out, lhsT, rhs, start, stop):
        nc.tensor.matmul(out, lhsT, rhs, start=start, stop=stop)

    ghv_v = ghvT.rearrange("t (k p) b -> t k p b", p=128)
    hA_v = hAT.rearrange("t w (k p) j -> t w k p j", p=128)
    edge_v = edge_d.rearrange("t (m p) b -> t m p b", p=128)
    eix_v = eidxT.rearrange("(k p) b -> k p b", p=128)
    enc_v = encT.rearrange("l (k p) b -> l k p b", p=128)

    with tile.TileContext(nc) as tc:
        # ---- Phase A1: edges.  edge = h@Wsh.T + v@Wsv.T for all rows, plus
        # a compacted correction h@(Wh-Wsh).T + v@(Wv-Wsv).T for the <=128
        # rows per step with t==0, scattered back into the psum group via a
        # 0/1 selection-matrix matmul.  corrT for step t+1 is computed (in
        # [j, h] orientation) during step t's main loop.
        with (
            tc.tile_pool(name="a1w", bufs=1) as wp,
            tc.tile_pool(name="a1hv", bufs=2) as hvp,
            tc.tile_pool(name="a1x", bufs=1) as xp,
            tc.tile_pool(name="a1ps", bufs=3, space="PSUM") as pp,
            tc.tile_pool(name="a1cps", bufs=2, space="PSUM") as cpp,
        ):
            wt = wp.tile([128, KH, 2, KD, 128], F32R)
            hvs, sts, has = {}, {}, {}

            def prefetch(t):
                if t >= NSTEP:
                    return
                ht = hvp.tile([128, KD, BS], F32R, tag="h")
                vt = hvp.tile([128, KD, BS], F32R, tag="v")
                for k in range(KD):
                    nc.sync.dma_start(ht[:, k, :], ghv_v[t, k])
                for k in range(KD):
                    nc.sync.dma_start(vt[:, k, :], ghv_v[t, KD + k])
                st = xp.tile([JC, BS], F32R, tag="smat")
                nc.sync.dma_start(st[:], smat[t])
                ha = xp.tile([128, 2, KD, JC], F32R, tag="hA")
                for w in range(2):
                    for k in range(KD):
                        nc.sync.dma_start(ha[:, w, k], hA_v[t, w, k])
                hvs[t], sts[t], has[t] = (ht, vt), st, ha

            # t=0 inputs by hand, interleaved so the first pb group's
            # operands (h0 + wS[0]) land first
            h0 = hvp.tile([128, KD, BS], F32R, tag="h")
            v0 = hvp.tile([128, KD, BS], F32R, tag="v")
            for k in range(KD):
                nc.sync.dma_start(h0[:, k, :], ghv_v[0, k])
            nc.sync.dma_start(wt[:, 0, 0], wS[0, :, 0])
            nc.sync.dma_start(wt[:, 0, 1], wS[0, :, 1])
            for k in range(KD):
                nc.sync.dma_start(v0[:, k, :], ghv_v[0, KD + k])
            nc.sync.dma_start(wt[:, 1, 0], wS[1, :, 0])
            nc.sync.dma_start(wt[:, 1, 1], wS[1, :, 1])
            wdt = wp.tile([128, 2, KD, H], F32R)
            for w in range(2):
                for k in range(KD):
                    nc.sync.dma_start(wdt[:, w, k, :], wD[w, k])
            s0 = xp.tile([JC, BS], F32R, tag="smat")
            nc.sync.dma_start(s0[:], smat[0])
            ha0 = xp.tile([128, 2, KD, JC], F32R, tag="hA")
            for w in range(2):
                for k in range(KD):
                    nc.sync.dma_start(ha0[:, w, k], hA_v[0, w, k])
            hvs[0], sts[0], has[0] = (h0, v0), s0, ha0
            for m in range(2, KH):
                for w in range(2):
                    nc.sync.dma_start(wt[:, m, w], wS[m, :, w])

            def corr_mms(t, ct):
                # corrT[j, h] for step t into 2 psum banks -> SBUF tile ct
                ha = has[t]
                for half in range(2):
                    cps = cpp.tile([JC, 512], F32, tag="cps")
                    for w in range(2):
                        for k in range(KD):
                            mm(cps[:], ha[:, w, k],
                               wdt[:, w, k, half * 512:(half + 1) * 512],
                               start=(w == 0 and k == 0),
                               stop=(w == 1 and k == KD - 1))
                    nc.vector.tensor_copy(
                        ct[:, half * 512:(half + 1) * 512], cps[:])

            ct = None
            for t in range(NSTEP):
                ht, vt = hvs[t]
                st = sts[t]
                prefetch(t + 1)
                ctn = None
                pend = []
                for m in range(KH):
                    pb = pp.tile([128, BS], F32, tag="pb")
                    for k in range(KD):
                        mm(pb[:], wt[:, m, 0, k, :], ht[:, k, :],
                           start=(k == 0), stop=False)
                    for k in range(KD):
                        mm(pb[:], wt[:, m, 1, k, :], vt[:, k, :],
                           start=False, stop=False)
                    if t == 0 and m < 2:
                        # defer the scatter so the PE isn't gated on the
                        # (large) wD prologue DMA for its first groups
                        pend.append((m, pb))
                        if m == 1:
                            ct = xp.tile([JC, H], F32R, tag="corrT")
                            corr_mms(0, ct)
                            for mp, pbp in pend:
                                mm(pbp[:],
                                   ct[:, mp * 128:(mp + 1) * 128], st[:],
                                   start=False, stop=True)
                                ot = xp.tile([128, BS], F32R, tag="ot")
                                nc.vector.tensor_copy(
                                    ot.bitcast(F32)[:], pbp[:])
                                nc.sync.dma_start(edge_v[t, mp], ot[:])
                        continue
                    mm(pb[:], ct[:, m * 128:(m + 1) * 128], st[:],
                       start=False, stop=True)
                    if m == 3 and t < NSTEP - 1:
                        ctn = xp.tile([JC, H], F32R, tag="corrT")
                        corr_mms(t + 1, ctn)
                    ot = xp.tile([128, BS], F32R, tag="ot")
                    nc.vector.tensor_copy(ot.bitcast(F32)[:], pb[:])
                    nc.sync.dma_start(edge_v[t, m], ot[:])
                if ctn is not None:
                    ct = ctn

        # inp spans through phase B (the tanh input shift)
        spanB_cm = tc.tile_pool(name="spanB", bufs=1)
        spanB = spanB_cm.__enter__()
        inps = spanB.tile([128, KH, BS], F32)

        # ---- Phase A2 (merged): cand_t = edge_t @ We.T;
        #   qt = relu(edge4 + relu(max_t cand_t));   lin = enc_idx @ Wi.T;
        #   q2 = relu(relu(qt + lin + bi) + lin + bi);  inp = q2 @ Wq.T
        with (
            tc.tile_pool(name="a2w", bufs=1) as wp,
            tc.tile_pool(name="a2e", bufs=2) as ep,
            tc.tile_pool(name="a2s", bufs=1) as sp,
            tc.tile_pool(name="a2t", bufs=1) as tp,
            tc.tile_pool(name="a2ps", bufs=2, space="PSUM") as pp,
        ):
            wt = wp.tile([128, KH, KD, 128], F32R)
            wit = wp.tile([128, KH, KD, 128], F32R)
            wqt = wp.tile([128, KH, KD, 128], F32R)
            eix = sp.tile([128, KD, BS], F32R)
            bi_t = sp.tile([128, KH], F32)
            nc.sync.dma_start(bi_t[:], bi[:])
            q2 = sp.tile([128, KH, BS], F32R)
            stq = sp.tile([128, KH, BS], F32)  # cand-max, then qt in place
            ests = []

            def est_load(t):
                e = ep.tile([128, KH, BS], F32R, tag="est")
                for m in range(KH):
                    nc.sync.dma_start(e[:, m, :], edge_v[t, m])
                ests.append(e)

            est_load(0)
            for m in range(KH):
                nc.sync.dma_start(wt[:, m], we4[m])
            est_load(1)
            for k in range(KD):
                nc.sync.dma_start(eix[:, k, :], eix_v[k])
            est_load(2)
            for m in range(KH):
                nc.sync.dma_start(wit[:, m], wi4[m])
            est_load(3)
            for m in range(KH):
                nc.sync.dma_start(wqt[:, m], wq4[m])
            est_load(4)
            for t in range(NSTEP):
                est = ests[t]
                for m in range(KH):
                    ps = pp.tile([128, BS], F32, tag="pc")
                    for k in range(KD):
                        mm(ps[:], wt[:, m, k, :], est[:, k, :],
                           start=(k == 0), stop=(k == KD - 1))
                    if t == 0:
                        nc.vector.tensor_copy(stq[:, m, :], ps[:])
                    else:
                        nc.vector.tensor_max(stq[:, m, :], stq[:, m, :],
                                             ps[:])
                    if t == NSTEP - 1:
                        nc.vector.tensor_relu(stq[:, m, :], stq[:, m, :])
                        nc.vector.tensor_add(
                            stq[:, m, :], est.bitcast(F32)[:, m, :],
                            stq[:, m, :])
                        nc.vector.tensor_relu(stq[:, m, :], stq[:, m, :])
            for m in range(KH):
                ps = pp.tile([128, BS], F32, tag="pl")
                for k in range(KD):
                    mm(ps[:], wit[:, m, k, :], eix[:, k, :],
                       start=(k == 0), stop=(k == KD - 1))
                lin = tp.tile([128, BS], F32, tag="lin")
                nc.vector.tensor_copy(lin[:], ps[:])
                t1 = tp.tile([128, BS], F32, tag="t1")
                nc.vector.tensor_add(t1[:], stq[:, m, :], lin[:])
                q1 = tp.tile([128, BS], F32, tag="q1")
                nc.scalar.activation(q1[:], t1[:], AF.Relu,
                                     bias=bi_t[:, m:m + 1])
                t2 = tp.tile([128, BS], F32, tag="t2")
                nc.vector.tensor_add(t2[:], q1[:], lin[:])
                nc.scalar.activation(q2[:, m, :], t2[:], AF.Relu,
                                     bias=bi_t[:, m:m + 1])
            for m in range(KH):
                ps = pp.tile([128, BS], F32, tag="pq")
                for k in range(KD):
                    mm(ps[:], wqt[:, m, k, :], q2[:, k, :],
                       start=(k == 0), stop=(k == KD - 1))
                nc.vector.tensor_copy(inps[:, m, :], ps[:])

        # ---- Phase B: attention.  For each l:
        #   ctx chunk = Wc-proj; y = tanh(ctx + inp + bias);
        #   att[k, b] += V4seg.T @ y  (accumulated over the 32 chunks)
        with (
            tc.tile_pool(name="bw", bufs=1) as wp,
            tc.tile_pool(name="be", bufs=2) as ep,
            tc.tile_pool(name="bt", bufs=3) as tp,
            tc.tile_pool(name="bps", bufs=4, space="PSUM") as pp,
            tc.tile_pool(name="baps", bufs=2, space="PSUM") as app,
        ):
            btot_t = wp.tile([128, NC4], F32)
            v4_t = wp.tile([128, NC4, 4], F32R)
            nc.sync.dma_start(btot_t[:], btot[:])
            nc.sync.dma_start(v4_t[:], v4s[:])
            et0 = ep.tile([128, KD, BS], F32R, tag="et")
            for k in range(KD):
                nc.sync.dma_start(et0[:, k, :], enc_v[0, k])
            wct = wp.tile([128, NC4, KD, 128], F32R)
            for c4 in range(NC4):
                nc.sync.dma_start(wct[:, c4], wc4[c4])
            et = et0
            for l in range(L):
                if l > 0:
                    et = ep.tile([128, KD, BS], F32R, tag="et")
                    for k in range(KD):
                        nc.sync.dma_start(et[:, k, :], enc_v[l, k])
                attps = app.tile([4, BS], F32, tag="attps")
                ys = []
                for c4 in range(NC4):
                    pc = pp.tile([128, BS], F32, tag="pctx")
                    for k in range(KD):
                        mm(pc[:], wct[:, c4, k, :], et[:, k, :],
                           start=(k == 0), stop=(k == KD - 1))
                    # att matmul for the previous chunk goes after this
                    # group so the PE never waits on DVE/ACT latency.
                    if ys:
                        c4p, yp = ys[-1]
                        mm(attps[:], v4_t[:, c4p, :], yp[:],
                           start=(c4p == 0), stop=False)
                    ypre = tp.tile([128, BS], F32, tag="ypre")
                    nc.vector.tensor_add(ypre[:], pc[:],
                                         inps[:, c4 % KH, :])
                    y = tp.tile([128, BS], F32R, tag="y")
                    nc.scalar.activation(y[:], ypre[:], AF.Tanh,
                                         bias=btot_t[:, c4:c4 + 1])
                    ys.append((c4, y))
                c4p, yp = ys[-1]
                mm(attps[:], v4_t[:, c4p, :], yp[:], start=False, stop=True)
                asb = tp.tile([4, BS], F32, tag="asb")
                nc.vector.tensor_copy(asb[:], attps[:])
                nc.sync.dma_start(att[:, l, :], asb[:])

        spanB_cm.__exit__(None, None, None)

    nc.finalize()
    return nc


def _get_program():
    global _PROG
    if _PROG is None:
        _PROG = _build_program()
    return _PROG


def _prep_inputs(encoder_output, xes, idx):
    """Build the 8 per-core input maps (all float32 numpy)."""
    enc = np.ascontiguousarray(np.asarray(encoder_output, dtype=np.float32))
    xes = np.asarray(xes)
    idx = np.asarray(idx)
    ar = np.arange(B)

    # [NCORES, L, D, BS]
    encT = np.ascontiguousarray(
        enc.reshape(NCORES, BS, L, D).transpose(0, 2, 3, 1))

    h = enc[ar[:, None], xes[:, :NSTEP, 0]]  # [B, 5, D]
    v = enc[ar[:, None], xes[:, :NSTEP, 1]]
    ghv = np.stack([h, v], axis=2)  # [B, 5, 2, D]
    ghvT = np.ascontiguousarray(
        ghv.reshape(NCORES, BS, NSTEP, 2 * D).transpose(0, 2, 3, 1))

    eidx = enc[ar, idx]  # [B, D]
    eidxT = np.ascontiguousarray(
        eidx.reshape(NCORES, BS, D).transpose(0, 2, 1))

    # compacted branch-A (t==0) rows per (core, step) + scatter matrix
    cond = (xes[:, :NSTEP, 2] == 0)  # [B, 5]
    hAT = np.zeros((NCORES, NSTEP, 2, D, JC), np.float32)
    smat = np.zeros((NCORES, NSTEP, JC, BS), np.float32)
    overflow = []
    for c in range(NCORES):
        for t in range(NSTEP):
            rows = np.nonzero(cond[c * BS:(c + 1) * BS, t])[0]
            if len(rows) > JC:
                overflow.extend((c * BS + rows[JC:]).tolist())
                rows = rows[:JC]
            n = len(rows)
            grows = c * BS + rows
            hAT[c, t, 0, :, :n] = h[grows, t].T
            hAT[c, t, 1, :, :n] = v[grows, t].T
            smat[c, t, np.arange(n), rows] = 1.0
    return encT, ghvT, eidxT, hAT, smat, np.array(overflow, np.int64)


def _prep_weights(Wq, bq, Wc, bc, V4, Wi, bi, Wh, Wv, Wsh, Wsv, We):
    f = lambda a: np.ascontiguousarray(np.asarray(a, dtype=np.float32))

    def pack(Wt):
        # W.T [d, h] -> [m, p, k, q]  (m = h chunk, k = d chunk)
        return np.ascontiguousarray(
            Wt.reshape(KD, 128, KH, 128).transpose(2, 1, 0, 3))

    wS = np.ascontiguousarray(np.stack(
        [pack(f(Wsh).T), pack(f(Wsv).T)], axis=2))  # [m, p, 2, k, q]
    wD = np.ascontiguousarray(np.stack([
        (f(Wh) - f(Wsh)).T.reshape(KD, 128, H),
        (f(Wv) - f(Wsv)).T.reshape(KD, 128, H)]))  # [2, k, p, h]
    we4 = pack(f(We).T)
    wi4 = pack(f(Wi).T)
    wq4 = pack(f(Wq).T)
    wcT = np.ascontiguousarray(f(Wc).transpose(2, 0, 1).reshape(D, 4 * H))
    wc4 = np.ascontiguousarray(
        wcT.reshape(KD, 128, NC4, 128).transpose(2, 1, 0, 3))  # [c4,p,k,q]
    bi_t = np.ascontiguousarray(f(bi).reshape(KH, 128).T)
    bcq = (f(bc) + f(bq)[None, :]).reshape(4 * H)  # bias for tanh input
    btot = np.ascontiguousarray(bcq.reshape(NC4, 128).T)
    v4s = np.zeros((128, NC4, 4), np.float32)
    V4f = f(V4)
    for c4 in range(NC4):
        k = c4 // KH
        v4s[:, c4, k] = V4f[k, (c4 % KH) * 128:(c4 % KH + 1) * 128]
    return dict(wS=wS, wD=wD, we4=we4, wi4=wi4, wq4=wq4, wc4=wc4,
                bi=bi_t, btot=btot, v4s=v4s)


def run_device(encoder_output, xes, idx, weights, trace=False,
               trace_cores=None):
    """Run the Bass kernel on 8 cores; returns (att_raw [B,4,L], overflow, res)."""
    from concourse.bass_utils import run_bass_kernel_spmd

    nc = _get_program()
    encT, ghvT, eidxT, hAT, smat, overflow = _prep_inputs(
        encoder_output, xes, idx)
    wmap = _prep_weights(**weights)
    in_maps = []
    for c in range(NCORES):
        m = {"encT": encT[c], "ghvT": ghvT[c], "eidxT": eidxT[c],
             "hAT": hAT[c], "smat": smat[c]}
        m.update(wmap)
        in_maps.append(m)
    res = run_bass_kernel_spmd(nc, in_maps, list(range(NCORES)),
                               trace=trace, trace_cores=trace_cores)
    att = np.stack([r["att"] for r in res.results])  # [8, 4, L, BS]
    att_raw = np.ascontiguousarray(
        att.transpose(0, 3, 1, 2).reshape(B, 4, L))
    return att_raw, overflow, res


def _edge_chain_host(enc, xes, idx, W, rows):
    """Exact recompute of att_raw for the given batch rows (BLAS float32)."""
    f = lambda a: np.asarray(a, dtype=np.float32)
    e = f(enc)[rows]  # [n, L, D]
    x = np.asarray(xes)[rows]
    n = len(rows)
    an = np.arange(n)
    WhT, WvT = f(W["Wh"]).T, f(W["Wv"]).T
    WshT, WsvT = f(W["Wsh"]).T, f(W["Wsv"]).T
    WeT, WiT, WqT = f(W["We"]).T, f(W["Wi"]).T, f(W["Wq"]).T
    Wc, bi, bq, bc, V4 = f(W["Wc"]), f(W["bi"]), f(W["bq"]), f(W["bc"]), f(W["V4"])

    el = np.zeros((n, H), np.float32)
    st = np.zeros((n, H), np.float32)
    qt = None
    for t in range(T):
        h = e[an, x[:, t, 0]]
        v = e[an, x[:, t, 1]]
        cond = (x[:, t, 2] == 0)[:, None]
        edge = np.where(cond, h @ WhT + v @ WvT, v @ WsvT + h @ WshT)
        subtree = np.maximum(st, edge @ WeT)
        qt = np.maximum(el + st, 0.0)
        el, st = edge, subtree
    enc_idx = e[an, np.asarray(idx)[rows]]
    lin = enc_idx @ WiT + bi
    q = np.maximum(qt + lin, 0.0)
    q = np.maximum(q + lin, 0.0)
    inp = q @ WqT + bq  # [n, H]
    WcT2 = np.ascontiguousarray(Wc.transpose(2, 0, 1).reshape(D, 4 * H))
    ctx = (e.reshape(n * L, D) @ WcT2).reshape(n, L, 4, H)
    y = np.tanh(inp[:, None, None, :] + ctx + bc[None, None, :, :])
    att_raw = np.einsum("nlkh,kh->nkl", y, V4, optimize=True)  # [n, 4, L]
    return att_raw.astype(np.float64)


def kernel(encoder_output, xes, idx, mask, Wq, bq, Wc, bc, V4, Wi, bi,
           Wh, Wv, Wsh, Wsv, We):
    import jax
    import jax.numpy as jnp

    enc = np.asarray(encoder_output, dtype=np.float32)
    xes = np.asarray(xes)
    idx = np.asarray(idx)
    mask = np.asarray(mask)
    weights = dict(Wq=Wq, bq=bq, Wc=Wc, bc=bc, V4=V4, Wi=Wi, bi=bi,
                   Wh=Wh, Wv=Wv, Wsh=Wsh, Wsv=Wsv, We=We)

    att_raw, overflow, _ = run_device(enc, xes, idx, weights)  # [B, 4, L]

    def finish(att_raw_f64):
        a = att_raw_f64.reshape(B, 4 * L)
        mask4 = np.tile(mask != 0, (1, 4))
        a = np.where(mask4, a, -np.inf)
        a = 10.0 * np.tanh(a)
        amax = a.max(axis=0)
        ex = np.exp(a - amax[None, :])
        s = ex.sum(axis=0)
        alpha = ex / s[None, :]
        lse = amax + np.log(s)
        logits = a - lse[None, :]
        return a, alpha, logits

    att64 = att_raw.astype(np.float64)
    a, alpha, logits = finish(att64)

    # gumbel noise — exactly what jax.random.categorical(key, logits,
    # axis=1) adds before its argmax
    G = np.asarray(jax.random.gumbel(jax.random.key(42), (B, 4 * L),
                                     jnp.float32), dtype=np.float64)
    pert = logits + G
    part = np.partition(pert, 4 * L - 2, axis=1)
    margin = part[:, -1] - part[:, -2]
    # rescue rows where the argmax could flip under the device's matmul
    # error, and rows whose selected probability is off the tanh
    # saturation plateau (where p inherits the raw att error)
    chosen0 = np.argmax(pert, axis=1)
    raw_sel = np.take_along_axis(att64.reshape(B, 4 * L), chosen0[:, None],
                                 axis=1)[:, 0]
    riskmask = (margin < DELTA) | (np.abs(raw_sel) < ASAT)
    if len(overflow):
        riskmask[overflow] = True
    risky = np.nonzero(riskmask)[0]
    if len(risky) > 0:
        att64[risky] = _edge_chain_host(enc, xes, idx, weights, risky)
        a, alpha, logits = finish(att64)
        pert = logits + G

    indices = np.argmax(pert, axis=1).astype(np.int32)[:, None]
    p = np.take_along_axis(alpha, indices, axis=1).astype(np.float32)
    one_hot = (np.arange(L)[None, :] == indices).astype(mask.dtype)
    mask_out = mask - one_hot
    return indices, p, mask_out
